# revision 14
# baseline (speedup 1.0000x reference)
"""MoFE (mixture of depthwise-conv experts) Trainium2 kernel, v2.

Full inputs in, full outputs out; internally sharded data-parallel over the
batch dim across 8 NeuronCores (B=8, one sample per core).

Per-core program (Bass/Tile):
  pass A: stream x; per block: ACT casts to resident fp8 plane x8 (+row sums),
          Pool computes fp8 residual r8 = x - x8 (interleaved with x8), DVE
          max-reduces. Gate (pooled -> fc -> noisy top-k softmax) on device.
  conv1:  PE fp8 DoubleRow matmuls: 9 taps on (x8, r8) pairs + 5 tap-paired
          delta-weight correction matmuls -> f32 psum; ACT relu(+bias) -> h.
  conv2:  27 taps split across lanes: DVE mul+add pairs, ACT muls (+DVE adds),
          DVE muls + gpsimd accumulate-DMA adds, Pool mul+add pairs.
  merge:  DVE add of the two partial accumulators; ACT bias+f32; DMA store.
"""

import numpy as np

import concourse.bass as bass
import concourse.tile as tile
from concourse import mybir
from concourse.ap import AP as _AP

F32 = mybir.dt.float32
BF = mybir.dt.bfloat16
F8 = mybir.dt.float8e4
AX = mybir.AxisListType if hasattr(mybir, "AxisListType") else None
ALU = mybir.AluOpType
ACT = mybir.ActivationFunctionType

B, C, H, W = 8, 96, 192, 192
E = 6
N_CORES = 8
TH = 24                      # strip height (output rows per strip)
NS = H // TH                 # strips
TAPS = [(ky - 1, kx - 1) for ky in range(3) for kx in range(3)]
K = 3                        # top-k slots
WP = W + 2                   # padded width
NPAD = WP * WP
GUARD = 200                  # pixels of guard around the padded image
HFLAT = (TH + 2) * WP        # flat pixels per conv1 strip (incl halo rows)
ACHUNK = 1024                # psum chunk (2 banks) read at once by ACT
MCHUNK = 512                 # matmul psum sub-chunk (1 bank)

# conv2 lane schedule: per slot a list of 9 lane codes
#   'd' = DVE mul + DVE add, 'a' = ACT mul + DVE add,
#   'm' = DVE mul + gpsimd accumulate-DMA add, 'p' = Pool mul + Pool add
LANES = [
    ['a', 'd', 'd', 'm', 'a', 'd', 'd', 'a', 'd'],
    ['a', 'd', 'd', 'm', 'a', 'd', 'p', 'a', 'd'],
    ['a', 'd', 'd', 'm', 'a', 'd', 'm', 'a', 'm'],
]


# ---------------------------------------------------------------------------
# walrus workaround: split instructions carrying >maxw semaphore waits
# ---------------------------------------------------------------------------
def _split_multiwait(nc, maxw: int = 1) -> int:
    n_split = 0
    for f in nc.m.functions:
        for b in f.blocks:
            insts = b.instructions
            new_list = []
            changed = False
            for inst in insts:
                si = getattr(inst, "sync_info", None)
                waits = list(si.on_wait) if (si and si.on_wait) else []
                if len(waits) > maxw:
                    changed = True
                    chunks = [waits[j: j + maxw] for j in range(0, len(waits), maxw)]
                    for k, ch in enumerate(chunks[:-1]):
                        nop = mybir.InstNoOp(
                            name=f"{inst.name}_wsplit{k}",
                            sync_info=mybir.SyncInfo(on_wait=ch, on_update=[]),
                            bass_nofuse=True,
                            engine=inst.engine,
                        )
                        new_list.append(nop)
                        n_split += 1
                    si.on_wait = chunks[-1]
                new_list.append(inst)
            if changed:
                if isinstance(insts, list):
                    insts[:] = new_list
                else:
                    b.instructions = new_list
    return n_split


def _raw_ap(base, elem_off, dims):
    """Build a raw strided AP from a 2D tile AP `base` ([C, L] view):
    keeps the partition dim, replaces free dims with `dims` ([stride, count])
    and offsets by `elem_off` elements."""
    ap0 = [list(d) for d in base.ap]
    part = ap0[0]
    return _AP(base.tensor, base.offset + elem_off,
               [part] + [list(d) for d in dims])


# ---------------------------------------------------------------------------
# device program
# ---------------------------------------------------------------------------
def _build(split: bool = True):
    nc = bass.Bass()
    x = nc.declare_dram_parameter("x", [C, H, W], F32, isOutput=False)
    wfc = nc.declare_dram_parameter("wfc", [C, 2 * E], F32, isOutput=False)
    bfc = nc.declare_dram_parameter("bfc", [1, 2 * E], F32, isOutput=False)
    w1 = nc.declare_dram_parameter("w1", [C, E * 9], F32, isOutput=False)
    b1 = nc.declare_dram_parameter("b1", [C, E], F32, isOutput=False)
    w2 = nc.declare_dram_parameter("w2", [C, E * 9], F32, isOutput=False)
    b2 = nc.declare_dram_parameter("b2", [C, E], F32, isOutput=False)
    eye = nc.declare_dram_parameter("eye", [C, C], F32, isOutput=False)
    eyeeye = nc.declare_dram_parameter("eyeeye", [C, 2 * C], F32, isOutput=False)
    y = nc.declare_dram_parameter("y", [C, H, W], F32, isOutput=True)

    v = nc.vector
    g = nc.gpsimd
    sc = nc.scalar
    sy = nc.sync

    DBI = 2 * GUARD              # element offset of pixel 0 in xi (interleaved)
    XLEN = 2 * GUARD + 2 * NPAD + 2 * GUARD

    with tile.TileContext(nc) as tc:
        with (
            tc.tile_pool(name="const", bufs=1) as cpool,
            tc.tile_pool(name="gate", bufs=1) as gpool,
            tc.tile_pool(name="xa", bufs=2) as xa_pool,
            tc.tile_pool(name="hbuf", bufs=2) as h_pool,
            tc.tile_pool(name="tmp1", bufs=1) as t1_pool,
            tc.tile_pool(name="tmp2", bufs=2) as t2_pool,
            tc.tile_pool(name="oacc", bufs=1) as oacc_pool,
            tc.tile_pool(name="of32", bufs=2) as of32_pool,
            tc.tile_pool(name="psg", bufs=1, space="PSUM") as psg_pool,
            tc.tile_pool(name="psc", bufs=3, space="PSUM") as psc_pool,
        ):
            # ---- constants ------------------------------------------------
            w1_sb = cpool.tile([C, E * 9], F32)
            sy.dma_start(w1_sb[:], w1[:])
            b1_sb = cpool.tile([C, E], F32)
            sy.dma_start(b1_sb[:], b1[:])
            w2_sb = cpool.tile([C, E * 9], F32)
            sy.dma_start(w2_sb[:], w2[:])
            b2_sb = cpool.tile([C, E], F32)
            sy.dma_start(b2_sb[:], b2[:])
            wfc_sb = cpool.tile([C, 2 * E], F32)
            sy.dma_start(wfc_sb[:], wfc[:])
            bfc_sb = cpool.tile([1, 2 * E], F32)
            sy.dma_start(bfc_sb[:], bfc[:])
            eye_sb = cpool.tile([C, C], F32)
            sy.dma_start(eye_sb[:], eye[:])
            ee_sb = cpool.tile([C, 2 * C], F32)
            sy.dma_start(ee_sb[:], eyeeye[:])
            ones96 = cpool.tile([1, C], F32)
            g.memset(ones96[:], 1.0)

            # resident interleaved fp8 image: even = x8, odd = r8 = x - x8
            xi = cpool.tile([C, XLEN], F8)
            # 4D interior view [c, row, col, plane] over the padded image
            xiI = xi[:, DBI:DBI + 2 * NPAD].rearrange(
                "c (r w two) -> c r w two", w=WP, two=2)
            # zero guards + padding ring (interleaved zeros are zeros)
            g.memset(xi[:, 0:DBI + 2 * WP], 0.0)
            g.memset(xi[:, DBI + 2 * (WP - 1) * WP:XLEN], 0.0)
            g.memset(xiI[:, 1:WP - 1, 0:1, :], 0.0)
            g.memset(xiI[:, 1:WP - 1, WP - 1:WP, :], 0.0)

            # ---- pass A: load f32; fp8 split; reduces ---------------------
            THA = 8
            NSA = H // THA
            maxbuf = gpool.tile([C, NSA], F32)
            sumbuf = gpool.tile([C, NSA], F32)
            for s in range(NSA):
                xa = xa_pool.tile([C, THA, W], F32)
                if s % 3 == 2:
                    g.dma_start(xa[:], x[:, s * THA:(s + 1) * THA, :])
                else:
                    sy.dma_start(xa[:], x[:, s * THA:(s + 1) * THA, :])
                x8v = xiI[:, 1 + s * THA:1 + (s + 1) * THA, 1:W + 1, 0]
                r8v = xiI[:, 1 + s * THA:1 + (s + 1) * THA, 1:W + 1, 1]
                sc.activation(x8v, xa[:], ACT.Copy, accum_out=sumbuf[:, s:s + 1])
                g.tensor_tensor(r8v, xa[:], x8v, ALU.subtract)
                v.tensor_reduce(maxbuf[:, s:s + 1], xa[:], AX.XY, ALU.max)
            maxv = gpool.tile([C, 1], F32)
            v.tensor_reduce(maxv[:], maxbuf[:], AX.X, ALU.max)
            sumv = gpool.tile([C, 1], F32)
            v.tensor_reduce(sumv[:], sumbuf[:], AX.X, ALU.add)
            pooled = gpool.tile([C, 1], F32)
            v.scalar_tensor_tensor(
                pooled[:], sumv[:], 1.0 / (H * W), maxv[:], ALU.mult, ALU.add
            )

            # ---- gate -----------------------------------------------------
            psg = psg_pool.tile([2 * E, 1], F32)
            nc.tensor.matmul(psg[:], wfc_sb[:], pooled[:], start=True, stop=True)
            g12 = gpool.tile([2 * E, 1], F32)
            v.tensor_copy(g12[:], psg[:])
            grow = gpool.tile([1, 2 * E], F32)
            sy.dma_start(grow[:], g12[:])          # partition -> free transpose
            gb = gpool.tile([1, 2 * E], F32)
            v.tensor_add(gb[:], grow[:], bfc_sb[:])
            g_pre = gb[:, 0:E]
            n_pre = gb[:, E:2 * E]

            # leaky relu(0.2)
            gl = gpool.tile([1, E], F32)
            t6 = gpool.tile([1, E], F32)
            v.tensor_scalar_mul(t6[:], g_pre, 0.2)
            v.tensor_max(gl[:], g_pre, t6[:])
            # softplus(x) = ln(1 + exp(x))
            e1 = gpool.tile([1, E], F32)
            sc.activation(e1[:], n_pre, ACT.Exp)
            noise = gpool.tile([1, E], F32)
            sc.activation(noise[:], e1[:], ACT.Ln, bias=1.0)
            # mean / unbiased std over experts
            mu = gpool.tile([1, 1], F32)
            v.tensor_reduce(mu[:], noise[:], AX.X, ALU.add)
            v.tensor_scalar_mul(mu[:], mu[:], 1.0 / E)
            d = gpool.tile([1, E], F32)
            v.tensor_scalar(d[:], noise[:], mu[:], None, ALU.subtract)
            dd = gpool.tile([1, E], F32)
            v.tensor_mul(dd[:], d[:], d[:])
            var = gpool.tile([1, 1], F32)
            v.tensor_reduce(var[:], dd[:], AX.X, ALU.add)
            v.tensor_scalar_mul(var[:], var[:], 1.0 / (E - 1))
            # 1/sqrt(var) via exp(-0.5 ln var) + one Newton step
            lnv = gpool.tile([1, 1], F32)
            sc.activation(lnv[:], var[:], ACT.Ln)
            isd0 = gpool.tile([1, 1], F32)
            sc.activation(isd0[:], lnv[:], ACT.Exp, scale=-0.5)
            ii = gpool.tile([1, 1], F32)
            v.tensor_mul(ii[:], isd0[:], isd0[:])
            v.tensor_mul(ii[:], ii[:], var[:])
            v.tensor_scalar(ii[:], ii[:], -0.5, 1.5, ALU.mult, ALU.add)
            isd = gpool.tile([1, 1], F32)
            v.tensor_mul(isd[:], isd0[:], ii[:])
            scores = gpool.tile([1, E], F32)
            v.scalar_tensor_tensor(scores[:], d[:], isd[:], gl[:], ALU.mult, ALU.add)

            # rank each expert
            ranks = gpool.tile([1, E], F32)
            cmp = gpool.tile([1, E], F32)
            for e in range(E):
                v.tensor_scalar(
                    cmp[:], scores[:], scores[0:1, e:e + 1], None, ALU.is_gt
                )
                v.tensor_reduce(ranks[:, e:e + 1], cmp[:], AX.X, ALU.add)
            mask = gpool.tile([1, E], F32)
            v.tensor_scalar(mask[:], ranks[:], float(K), None, ALU.is_lt)

            # softmax over selected: gm = (gl+30)*mask - 30
            gm = gpool.tile([1, E], F32)
            v.scalar_tensor_tensor(gm[:], gl[:], 30.0, mask[:], ALU.add, ALU.mult)
            v.tensor_scalar_sub(gm[:], gm[:], 30.0)
            gmax = gpool.tile([1, 1], F32)
            v.tensor_reduce(gmax[:], gm[:], AX.X, ALU.max)
            ngmax = gpool.tile([1, 1], F32)
            v.tensor_scalar_mul(ngmax[:], gmax[:], -1.0)
            ex = gpool.tile([1, E], F32)
            sc.activation(ex[:], gm[:], ACT.Exp, bias=ngmax[:])
            ssum = gpool.tile([1, 1], F32)
            v.tensor_reduce(ssum[:], ex[:], AX.X, ALU.add)
            rs = gpool.tile([1, 1], F32)
            v.reciprocal(rs[:], ssum[:])

            # cat = [cof(6) | slotmask(18) | cof*slotmask(18)] on partition 0
            cat = gpool.tile([1, E + 2 * K * E], F32)
            cof = cat[:, 0:E]
            v.tensor_scalar(cof, ex[:], rs[:], None, ALU.mult)
            for s in range(K):
                sm = cat[:, E + s * E:E + (s + 1) * E]
                v.tensor_scalar(sm, ranks[:], float(s), None, ALU.is_equal)
                v.tensor_mul(cat[:, E + K * E + s * E:E + K * E + (s + 1) * E], sm, cof)

            # broadcast cat to all partitions via ones[1,C].T @ cat[1,42]
            ps_bc = psg_pool.tile([C, E + 2 * K * E], F32, tag="bc")
            nc.tensor.matmul(ps_bc[:], ones96[:], cat[:], start=True, stop=True)
            bc = cpool.tile([C, E + 2 * K * E], F32)
            v.tensor_copy(bc[:], ps_bc[:])

            def smask_ap(s, e):      # slot-mask broadcast column
                return bc[:, E + s * E + e:E + s * E + e + 1]

            def csmask_ap(s, e):     # cof * slot-mask broadcast column
                return bc[:, E + K * E + s * E + e:E + K * E + s * E + e + 1]

            # gather slot weights: w1s (f32), w2s (f32, cof-scaled), b1s
            w1s = cpool.tile([C, K * 9], F32)
            w2s = cpool.tile([C, K * 9], F32)
            b1s = cpool.tile([C, K], F32)
            for s in range(K):
                for e in range(E):
                    i0 = w1_sb[:, e * 9:(e + 1) * 9]
                    o0 = w1s[:, s * 9:(s + 1) * 9]
                    if e == 0:
                        v.tensor_scalar(o0, i0, smask_ap(s, e), None, ALU.mult)
                    else:
                        v.scalar_tensor_tensor(o0, i0, smask_ap(s, e), o0,
                                               ALU.mult, ALU.add)
                    i2 = w2_sb[:, e * 9:(e + 1) * 9]
                    o2 = w2s[:, s * 9:(s + 1) * 9]
                    if e == 0:
                        v.tensor_scalar(o2, i2, csmask_ap(s, e), None, ALU.mult)
                    else:
                        v.scalar_tensor_tensor(o2, i2, csmask_ap(s, e), o2,
                                               ALU.mult, ALU.add)
                    ib = b1_sb[:, e:e + 1]
                    ob = b1s[:, s:s + 1]
                    if e == 0:
                        v.tensor_scalar(ob, ib, smask_ap(s, e), None, ALU.mult)
                    else:
                        v.scalar_tensor_tensor(ob, ib, smask_ap(s, e), ob,
                                               ALU.mult, ALU.add)
            # b2tot = sum_e cof_e * b2_e
            tb = gpool.tile([C, E], F32)
            v.tensor_mul(tb[:], b2_sb[:], bc[:, 0:E])
            b2tot = cpool.tile([C, 1], F32)
            v.tensor_reduce(b2tot[:], tb[:], AX.X, ALU.add)

            # ---- fp8 weight split: w1s = w8 + dw8 (both exactly fp8) ------
            w8q = cpool.tile([C, K * 9], F8)
            sc.activation(w8q[:], w1s[:], ACT.Copy)
            w8f = cpool.tile([C, K * 9], F32)
            sc.activation(w8f[:], w8q[:], ACT.Copy)
            dw8q = cpool.tile([C, K * 9], F8)
            v.tensor_tensor(dw8q[:], w1s[:], w8f[:], ALU.subtract)
            dw8f = cpool.tile([C, K * 9], F32)
            sc.activation(dw8f[:], dw8q[:], ACT.Copy)

            # DoubleRow stationary matrices.
            # main (per slot, tap): [C, 2, C] fp8; plane0 = plane1 = diag(w8)
            # delta (per slot, tap-pair): plane0 = diag(dw8[t0]), plane1 =
            #   diag(dw8[t1]) (or zero for the unpaired 9th tap)
            PAIRS = [(0, 1), (2, 3), (4, 5), (6, 7), (8, None)]
            dr_main = []
            dr_delta = []
            for s in range(K):
                row_m = []
                row_d = []
                for t in range(9):
                    m = cpool.tile([C, 2 * C], F8, tag=f"drm{s}_{t}")
                    v.tensor_scalar(m[:], ee_sb[:], w8f[:, 9 * s + t:9 * s + t + 1],
                                    None, ALU.mult)
                    row_m.append(m)
                for pi, (t0, t1) in enumerate(PAIRS):
                    dm = cpool.tile([C, 2 * C], F8, tag=f"drd{s}_{pi}")
                    v.tensor_scalar(dm[:, 0:C], eye_sb[:],
                                    dw8f[:, 9 * s + t0:9 * s + t0 + 1],
                                    None, ALU.mult)
                    if t1 is None:
                        g.memset(dm[:, C:2 * C], 0.0)
                    else:
                        v.tensor_scalar(dm[:, C:2 * C], eye_sb[:],
                                        dw8f[:, 9 * s + t1:9 * s + t1 + 1],
                                        None, ALU.mult)
                    row_d.append(dm)
                dr_main.append(row_m)
                dr_delta.append(row_d)

            DELTAS = [dy * WP + dx for (dy, dx) in TAPS]

            # ---- pass B ---------------------------------------------------
            # Software-pipelined emission: conv2 ops of slot k are emitted
            # interleaved between conv1 psum-chunks of slot k+1 so the ACT
            # relu chain (which gates PE via psum buffers) is never stuck
            # behind a burst of conv2 muls, and DVE adds trail their ACT
            # producers.
            xi_flat = xi[:]
            achunks = []
            a0 = 0
            while a0 < HFLAT:
                achunks.append((a0, min(ACHUNK, HFLAT - a0)))
                a0 += ACHUNK

            from collections import deque
            pend = deque()
            pend_late = deque()

            def _late_ready():
                return pend_late and pend_late[0][0]()

            def drain_mix(n_now, n_late):
                for _ in range(n_now):
                    if pend:
                        pend.popleft()()
                for _ in range(n_late):
                    if _late_ready():
                        pend_late.popleft()[1]()

            def drain_all():
                while pend or pend_late:
                    if pend:
                        pend.popleft()()
                    if _late_ready():
                        pend_late.popleft()[1]()

            def emit_conv1_chunk(slot, h0, hf, c0, csz):
                ps = psc_pool.tile([C, ACHUNK], F32, tag="convps")
                for m0 in range(0, csz, MCHUNK):
                    msz = min(MCHUNK, csz - m0)
                    pix0 = h0 * WP + c0 + m0
                    out_ap = ps[:, m0:m0 + msz]
                    for t in range(9):
                        off = DBI + 2 * (pix0 + DELTAS[t])
                        rhs = xi_flat[:, off:off + 2 * msz].rearrange(
                            "c (n two) -> c two n", two=2)
                        nc.tensor.matmul(
                            out_ap,
                            dr_main[slot][t][:].rearrange("c (i j) -> c i j", i=2),
                            rhs,
                            start=(t == 0), stop=False,
                            perf_mode=mybir.MatmulPerfMode.DoubleRow,
                        )
                    for pi, (t0, t1) in enumerate(PAIRS):
                        d0 = DELTAS[t0]
                        dgap = 0 if t1 is None else (DELTAS[t1] - d0)
                        rhs = _raw_ap(xi_flat, DBI + 2 * (pix0 + d0),
                                      [[2 * dgap, 2], [2, msz]])
                        nc.tensor.matmul(
                            out_ap,
                            dr_delta[slot][pi][:].rearrange("c (i j) -> c i j", i=2),
                            rhs,
                            start=False, stop=(pi == len(PAIRS) - 1),
                            perf_mode=mybir.MatmulPerfMode.DoubleRow,
                        )
                sc.activation(hf[:, c0:c0 + csz], ps[:, 0:csz],
                              ACT.Relu, bias=b1s[:, slot:slot + 1])

            state = {}

            def queue_conv2(s, slot, ht):
                ctr = state["ctr"]
                st = state["st"]
                def dec(f):
                    def g():
                        f()
                        ctr[0] -= 1
                    return g
                for it, (dy, dx) in enumerate(TAPS):
                    in0 = ht[:, 1 + dy:1 + dy + TH, 1 + dx:1 + dx + W]
                    wap = w2s[:, slot * 9 + it:slot * 9 + it + 1]
                    lane = LANES[slot][it]
                    oacc, oaccp = state["oacc"], state["oaccp"]
                    if lane == 'd':
                        def f(in0=in0, wap=wap, oacc=oacc, st=st):
                            if st.pop("fd", None):
                                v.tensor_scalar(oacc[:], in0, wap, None, ALU.mult)
                            else:
                                p = t2_pool.tile([C, TH, W], BF, tag="tm")
                                v.tensor_scalar(p[:], in0, wap, None, ALU.mult)
                                v.tensor_add(oacc[:], oacc[:], p[:])
                        pend.append(dec(f))
                    elif lane == 'a':
                        box = {}
                        def fm(in0=in0, wap=wap, box=box):
                            p = t2_pool.tile([C, TH, W], BF, tag="ta")
                            sc.activation(p[:], in0, ACT.Copy, scale=wap)
                            box["p"] = p
                        def fa(box=box, oaccp=oaccp, st=st):
                            if st.pop("fp", None):
                                g.dma_start(oaccp[:], box["p"][:])
                            else:
                                g.dma_start(oaccp[:], box["p"][:],
                                            accum_op=ALU.add)
                        pend.append(fm)
                        pend_late.append((lambda box=box: "p" in box, dec(fa)))
                    elif lane == 'p':
                        def f(in0=in0, wap=wap, oaccp=oaccp, st=st):
                            if st.pop("fp", None):
                                g.tensor_scalar(oaccp[:], in0, wap, None, ALU.mult)
                            else:
                                p = t1_pool.tile([C, TH, W], BF, tag="ts")
                                g.tensor_scalar(p[:], in0, wap, None, ALU.mult)
                                g.tensor_add(oaccp[:], oaccp[:], p[:])
                        pend.append(dec(f))
                    else:  # 'm'
                        def f(in0=in0, wap=wap, oaccp=oaccp, st=st):
                            p = t2_pool.tile([C, TH, W], BF, tag="tm")
                            v.tensor_scalar(p[:], in0, wap, None, ALU.mult)
                            if st.pop("fp", None):
                                g.dma_start(oaccp[:], p[:])
                            else:
                                g.dma_start(oaccp[:], p[:], accum_op=ALU.add)
                        pend.append(dec(f))

            def queue_finish(s, h0):
                oacc, oaccp = state["oacc"], state["oaccp"]
                ctr = state["ctr"]
                def fmerge(oacc=oacc, oaccp=oaccp):
                    v.tensor_add(oacc[:], oacc[:], oaccp[:])
                pend_late.append((lambda ctr=ctr: ctr[0] == 0, fmerge))
                HH = TH // 2
                for hb in range(2):
                    def ffin(hb=hb, h0=h0, oacc=oacc):
                        of32 = of32_pool.tile([C, HH, W], F32, tag="of32")
                        sc.activation(of32[:], oacc[:, hb * HH:(hb + 1) * HH, :],
                                      ACT.Identity, bias=b2tot[:])
                        sy.dma_start(y[:, h0 + hb * HH:h0 + (hb + 1) * HH, :],
                                     of32[:])
                    pend_late.append((lambda ctr=ctr: ctr[0] == 0, ffin))

            for s in range(NS):
                h0 = s * TH
                state["oacc"] = oacc_pool.tile([C, TH, W], BF, tag="oacc_dve", name="oacc")
                state["oaccp"] = oacc_pool.tile([C, TH, W], BF, tag="oacc_pool", name="oaccp")
                state["st"] = {"fd": True, "fp": True}
                state["ctr"] = [27]
                for slot in range(K):
                    ht = h_pool.tile([C, TH + 2, WP], BF)
                    hf = ht[:].rearrange("c r w -> c (r w)")
                    for (c0, csz) in achunks:
                        emit_conv1_chunk(slot, h0, hf, c0, csz)
                        drain_mix(2, 1)
                    drain_mix(1, 1)
                    # zero h padding (cols, and top/bottom edge rows)
                    g.memset(ht[:, :, 0:1], 0.0)
                    g.memset(ht[:, :, WP - 1:WP], 0.0)
                    if s == 0:
                        g.memset(ht[:, 0:1, :], 0.0)
                    if s == NS - 1:
                        g.memset(ht[:, TH + 1:TH + 2, :], 0.0)
                    queue_conv2(s, slot, ht)
                queue_finish(s, h0)
                while len(pend) + len(pend_late) > 14:
                    drain_mix(1, 1)
            drain_all()

    if split:
        _split_multiwait(nc, maxw=1)
    return nc


_NC_CACHE = {}


def _get_nc():
    if "nc" not in _NC_CACHE:
        _NC_CACHE["nc"] = _build()
    return _NC_CACHE["nc"]


class _Runner:
    """Compile-once SPMD runner (mirrors bass2jax.run_bass_via_pjrt's
    multi-core path, but keeps the jitted executable for reuse/benching)."""

    def __init__(self, nc, n_cores):
        import jax
        from jax.experimental.shard_map import shard_map
        from jax.sharding import Mesh, PartitionSpec
        from concourse import bass2jax, mybir as _mybir

        bass2jax.install_neuronx_cc_hook()
        self.jax = jax
        partition_name = (
            nc.partition_id_tensor.name if nc.partition_id_tensor else None
        )
        in_names, out_names, out_avals, zero_outs = [], [], [], []
        for alloc in nc.m.functions[0].allocations:
            if not isinstance(alloc, _mybir.MemoryLocationSet):
                continue
            name = alloc.memorylocations[0].name
            if alloc.kind == "ExternalInput":
                if name == partition_name:
                    continue
                in_names.append(name)
            elif alloc.kind == "ExternalOutput":
                shape = tuple(alloc.tensor_shape)
                dtype = _mybir.dt.np(alloc.dtype)
                out_names.append(name)
                out_avals.append(jax.core.ShapedArray(shape, dtype))
                zero_outs.append(np.zeros(shape, dtype))
        self.in_names, self.out_names = in_names, out_names
        self.out_avals, self.zero_outs = out_avals, zero_outs
        n_params, n_outs = len(in_names), len(out_names)
        self.n_cores = n_cores
        donate = tuple(range(n_params, n_params + n_outs))

        all_in_names = in_names + out_names
        if partition_name is not None:
            all_in_names = all_in_names + [partition_name]

        def _body(*args):
            operands = list(args)
            if partition_name is not None:
                operands.append(bass2jax.partition_id_tensor())
            outs = bass2jax._bass_exec_p.bind(
                *operands,
                out_avals=tuple(out_avals),
                in_names=tuple(all_in_names),
                out_names=tuple(out_names),
                lowering_input_output_aliases=(),
                sim_require_finite=True,
                sim_require_nnan=True,
                nc=nc,
            )
            return tuple(outs)

        devices = jax.devices()[:n_cores]
        mesh = Mesh(np.asarray(devices), ("core",))
        self.sharded = jax.jit(
            shard_map(
                _body,
                mesh=mesh,
                in_specs=(PartitionSpec("core"),) * (n_params + n_outs),
                out_specs=(PartitionSpec("core"),) * n_outs,
                check_rep=False,
            ),
            donate_argnums=donate,
            keep_unused=True,
        )

    def concat_inputs(self, in_maps):
        return [
            np.concatenate([np.asarray(m[name]) for m in in_maps], axis=0)
            for name in self.in_names
        ]

    def concat_zeros(self):
        return [
            np.zeros((self.n_cores * z.shape[0], *z.shape[1:]), z.dtype)
            for z in self.zero_outs
        ]

    def run(self, in_maps):
        out_arrs = self.sharded(*self.concat_inputs(in_maps), *self.concat_zeros())
        return [
            {
                name: np.asarray(out_arrs[i]).reshape(
                    self.n_cores, *self.out_avals[i].shape
                )[c]
                for i, name in enumerate(self.out_names)
            }
            for c in range(self.n_cores)
        ]


def _get_runner():
    if "runner" not in _NC_CACHE:
        _NC_CACHE["runner"] = _Runner(_get_nc(), N_CORES)
    return _NC_CACHE["runner"]


_EYE = np.ascontiguousarray(np.eye(C, dtype=np.float32))
_EYEEYE = np.ascontiguousarray(np.concatenate([_EYE, _EYE], axis=1))


def make_in_maps(x, w_fc0, b_fc0, w_fc1, b_fc1, ew1, eb1, ew2, eb2):
    x = np.asarray(x, dtype=np.float32)
    f32 = lambda a: np.ascontiguousarray(np.asarray(a, dtype=np.float32))
    wfc = f32(np.concatenate([np.asarray(w_fc1).T, np.asarray(w_fc0).T], axis=1))
    bfc = f32(np.concatenate([np.asarray(b_fc1), np.asarray(b_fc0)])[None, :])
    w1p = f32(np.asarray(ew1).reshape(E, C, 9).transpose(1, 0, 2).reshape(C, E * 9))
    b1p = f32(np.asarray(eb1).T)
    w2p = f32(np.asarray(ew2).reshape(E, C, 9).transpose(1, 0, 2).reshape(C, E * 9))
    b2p = f32(np.asarray(eb2).T)

    in_maps = []
    for b in range(B):
        in_maps.append({
            "x": np.ascontiguousarray(x[b]),
            "wfc": wfc, "bfc": bfc,
            "w1": w1p, "b1": b1p, "w2": w2p, "b2": b2p,
            "eye": _EYE, "eyeeye": _EYEEYE,
        })
    return in_maps


def kernel(x, w_fc0, b_fc0, w_fc1, b_fc1, ew1, eb1, ew2, eb2):
    in_maps = make_in_maps(x, w_fc0, b_fc0, w_fc1, b_fc1, ew1, eb1, ew2, eb2)
    res = _get_runner().run(in_maps)
    out = np.stack([res[b]["y"] for b in range(B)], axis=0)
    return out.astype(np.float32)


if __name__ == "__main__":
    data = np.load("/tmp/ref_data.npz")
    inputs = {k: data[k] for k in
              ["x", "w_fc0", "b_fc0", "w_fc1", "b_fc1", "ew1", "eb1", "ew2", "eb2"]}
    out = kernel(**inputs)
    exp = data["out"]
    err = np.linalg.norm(out - exp) / np.linalg.norm(exp)
    print("Relative error:", err)
    print("max abs diff:", np.abs(out - exp).max())


# revision 17
# speedup vs baseline: 1.0532x; 1.0532x over previous
"""MoFE (mixture of depthwise-conv experts) Trainium2 kernel, v2.

Full inputs in, full outputs out; internally sharded data-parallel over the
batch dim across 8 NeuronCores (B=8, one sample per core).

Per-core program (Bass/Tile):
  pass A: stream x; per block: ACT casts to resident fp8 plane x8 (+row sums),
          Pool computes fp8 residual r8 = x - x8 (interleaved with x8), DVE
          max-reduces. Gate (pooled -> fc -> noisy top-k softmax) on device.
  conv1:  PE fp8 DoubleRow matmuls: 9 taps on (x8, r8) pairs + 5 tap-paired
          delta-weight correction matmuls -> f32 psum; ACT relu(+bias) -> h.
  conv2:  27 taps split across lanes: DVE mul+add pairs, ACT muls (+DVE adds),
          DVE muls + gpsimd accumulate-DMA adds, Pool mul+add pairs.
  merge:  DVE add of the two partial accumulators; ACT bias+f32; DMA store.
"""

import numpy as np

import concourse.bass as bass
import concourse.tile as tile
from concourse import mybir
from concourse.ap import AP as _AP

F32 = mybir.dt.float32
BF = mybir.dt.bfloat16
F8 = mybir.dt.float8e4
AX = mybir.AxisListType if hasattr(mybir, "AxisListType") else None
ALU = mybir.AluOpType
ACT = mybir.ActivationFunctionType

B, C, H, W = 8, 96, 192, 192
E = 6
N_CORES = 8
TH = 24                      # strip height (output rows per strip)
NS = H // TH                 # strips
TAPS = [(ky - 1, kx - 1) for ky in range(3) for kx in range(3)]
K = 3                        # top-k slots
WP = W + 2                   # padded width
NPAD = WP * WP
GUARD = 200                  # pixels of guard around the padded image
HFLAT = (TH + 2) * WP        # flat pixels per conv1 strip (incl halo rows)
ACHUNK = 1024                # psum chunk (2 banks) read at once by ACT
MCHUNK = 512                 # matmul psum sub-chunk (1 bank)

# conv2 lane schedule: per slot a list of 9 lane codes
#   'd' = DVE mul + DVE add, 'a' = ACT mul + DVE add,
#   'm' = DVE mul + gpsimd accumulate-DMA add, 'p' = Pool mul + Pool add
LANES = [
    ['a', 'd', 'd', 'm', 'a', 'd', 'd', 'a', 'd'],
    ['a', 'd', 'd', 'm', 'a', 'd', 'p', 'a', 'd'],
    ['a', 'd', 'd', 'm', 'a', 'd', 'm', 'a', 'm'],
]


# ---------------------------------------------------------------------------
# walrus workaround: split instructions carrying >maxw semaphore waits
# ---------------------------------------------------------------------------
def _split_multiwait(nc, maxw: int = 1) -> int:
    n_split = 0
    for f in nc.m.functions:
        for b in f.blocks:
            insts = b.instructions
            new_list = []
            changed = False
            for inst in insts:
                si = getattr(inst, "sync_info", None)
                waits = list(si.on_wait) if (si and si.on_wait) else []
                if len(waits) > maxw:
                    changed = True
                    chunks = [waits[j: j + maxw] for j in range(0, len(waits), maxw)]
                    for k, ch in enumerate(chunks[:-1]):
                        nop = mybir.InstNoOp(
                            name=f"{inst.name}_wsplit{k}",
                            sync_info=mybir.SyncInfo(on_wait=ch, on_update=[]),
                            bass_nofuse=True,
                            engine=inst.engine,
                        )
                        new_list.append(nop)
                        n_split += 1
                    si.on_wait = chunks[-1]
                new_list.append(inst)
            if changed:
                if isinstance(insts, list):
                    insts[:] = new_list
                else:
                    b.instructions = new_list
    return n_split


def _raw_ap(base, elem_off, dims):
    """Build a raw strided AP from a 2D tile AP `base` ([C, L] view):
    keeps the partition dim, replaces free dims with `dims` ([stride, count])
    and offsets by `elem_off` elements."""
    ap0 = [list(d) for d in base.ap]
    part = ap0[0]
    return _AP(base.tensor, base.offset + elem_off,
               [part] + [list(d) for d in dims])


# ---------------------------------------------------------------------------
# device program
# ---------------------------------------------------------------------------
def _build(split: bool = True):
    nc = bass.Bass()
    x = nc.declare_dram_parameter("x", [C, H, W], F32, isOutput=False)
    wfc = nc.declare_dram_parameter("wfc", [C, 2 * E], F32, isOutput=False)
    bfc = nc.declare_dram_parameter("bfc", [1, 2 * E], F32, isOutput=False)
    w1 = nc.declare_dram_parameter("w1", [C, E * 9], F32, isOutput=False)
    b1 = nc.declare_dram_parameter("b1", [C, E], F32, isOutput=False)
    w2 = nc.declare_dram_parameter("w2", [C, E * 9], F32, isOutput=False)
    b2 = nc.declare_dram_parameter("b2", [C, E], F32, isOutput=False)
    eye = nc.declare_dram_parameter("eye", [C, C], F32, isOutput=False)
    eyeeye = nc.declare_dram_parameter("eyeeye", [C, 2 * C], F32, isOutput=False)
    y = nc.declare_dram_parameter("y", [C, H, W], F32, isOutput=True)

    v = nc.vector
    g = nc.gpsimd
    sc = nc.scalar
    sy = nc.sync

    DBI = 2 * GUARD              # element offset of pixel 0 in xi (interleaved)
    XLEN = 2 * GUARD + 2 * NPAD + 2 * GUARD

    with tile.TileContext(nc) as tc:
        with (
            tc.tile_pool(name="const", bufs=1) as cpool,
            tc.tile_pool(name="gate", bufs=1) as gpool,
            tc.tile_pool(name="xa", bufs=3) as xa_pool,
            tc.tile_pool(name="hbuf", bufs=2) as h_pool,
            tc.tile_pool(name="tmp2", bufs=2) as t2_pool,
            tc.tile_pool(name="oacc", bufs=1) as oacc_pool,
            tc.tile_pool(name="of32", bufs=2) as of32_pool,
            tc.tile_pool(name="psg", bufs=1, space="PSUM") as psg_pool,
            tc.tile_pool(name="psc", bufs=3, space="PSUM") as psc_pool,
        ):
            # ---- constants ------------------------------------------------
            w1_sb = cpool.tile([C, E * 9], F32)
            sy.dma_start(w1_sb[:], w1[:])
            b1_sb = cpool.tile([C, E], F32)
            sy.dma_start(b1_sb[:], b1[:])
            w2_sb = cpool.tile([C, E * 9], F32)
            sy.dma_start(w2_sb[:], w2[:])
            b2_sb = cpool.tile([C, E], F32)
            sy.dma_start(b2_sb[:], b2[:])
            wfc_sb = cpool.tile([C, 2 * E], F32)
            sy.dma_start(wfc_sb[:], wfc[:])
            bfc_sb = cpool.tile([1, 2 * E], F32)
            sy.dma_start(bfc_sb[:], bfc[:])
            eye_sb = cpool.tile([C, C], F32)
            sy.dma_start(eye_sb[:], eye[:])
            ee_sb = cpool.tile([C, 2 * C], F32)
            sy.dma_start(ee_sb[:], eyeeye[:])
            ones96 = cpool.tile([1, C], F32)
            g.memset(ones96[:], 1.0)

            # resident interleaved fp8 image: even = x8, odd = r8 = x - x8
            xi = cpool.tile([C, XLEN], F8)
            # 4D interior view [c, row, col, plane] over the padded image
            xiI = xi[:, DBI:DBI + 2 * NPAD].rearrange(
                "c (r w two) -> c r w two", w=WP, two=2)
            # zero guards + padding ring (interleaved zeros are zeros)
            g.memset(xi[:, 0:DBI + 2 * WP], 0.0)
            g.memset(xi[:, DBI + 2 * (WP - 1) * WP:XLEN], 0.0)
            g.memset(xiI[:, 1:WP - 1, 0:1, :], 0.0)
            g.memset(xiI[:, 1:WP - 1, WP - 1:WP, :], 0.0)

            # ---- pass A: load f32; fp8 split; reduces ---------------------
            THA = 8
            NSA = H // THA
            maxbuf = gpool.tile([C, NSA], F32)
            sumbuf = gpool.tile([C, NSA], F32)
            for s in range(NSA):
                xa = xa_pool.tile([C, THA, W], F32)
                sy.dma_start(xa[:], x[:, s * THA:(s + 1) * THA, :])
                x8v = xiI[:, 1 + s * THA:1 + (s + 1) * THA, 1:W + 1, 0]
                r8v = xiI[:, 1 + s * THA:1 + (s + 1) * THA, 1:W + 1, 1]
                sc.activation(x8v, xa[:], ACT.Copy, accum_out=sumbuf[:, s:s + 1])
                # Pool recomputes its own fp8 cast so r8 does not wait on ACT
                t8 = xa_pool.tile([C, THA, W], F8, tag="t8")
                g.tensor_copy(t8[:], xa[:])
                g.tensor_tensor(r8v, xa[:], t8[:], ALU.subtract)
                v.tensor_reduce(maxbuf[:, s:s + 1], xa[:], AX.XY, ALU.max)
            maxv = gpool.tile([C, 1], F32)
            v.tensor_reduce(maxv[:], maxbuf[:], AX.X, ALU.max)
            sumv = gpool.tile([C, 1], F32)
            v.tensor_reduce(sumv[:], sumbuf[:], AX.X, ALU.add)
            pooled = gpool.tile([C, 1], F32)
            v.scalar_tensor_tensor(
                pooled[:], sumv[:], 1.0 / (H * W), maxv[:], ALU.mult, ALU.add
            )

            # ---- gate -----------------------------------------------------
            psg = psg_pool.tile([2 * E, 1], F32)
            nc.tensor.matmul(psg[:], wfc_sb[:], pooled[:], start=True, stop=True)
            g12 = gpool.tile([2 * E, 1], F32)
            v.tensor_copy(g12[:], psg[:])
            grow = gpool.tile([1, 2 * E], F32)
            sy.dma_start(grow[:], g12[:])          # partition -> free transpose
            gb = gpool.tile([1, 2 * E], F32)
            v.tensor_add(gb[:], grow[:], bfc_sb[:])
            g_pre = gb[:, 0:E]
            n_pre = gb[:, E:2 * E]

            # leaky relu(0.2)
            gl = gpool.tile([1, E], F32)
            t6 = gpool.tile([1, E], F32)
            v.tensor_scalar_mul(t6[:], g_pre, 0.2)
            v.tensor_max(gl[:], g_pre, t6[:])
            # softplus(x) = ln(1 + exp(x))
            e1 = gpool.tile([1, E], F32)
            sc.activation(e1[:], n_pre, ACT.Exp)
            noise = gpool.tile([1, E], F32)
            sc.activation(noise[:], e1[:], ACT.Ln, bias=1.0)
            # mean / unbiased std over experts
            mu = gpool.tile([1, 1], F32)
            v.tensor_reduce(mu[:], noise[:], AX.X, ALU.add)
            v.tensor_scalar_mul(mu[:], mu[:], 1.0 / E)
            d = gpool.tile([1, E], F32)
            v.tensor_scalar(d[:], noise[:], mu[:], None, ALU.subtract)
            dd = gpool.tile([1, E], F32)
            v.tensor_mul(dd[:], d[:], d[:])
            var = gpool.tile([1, 1], F32)
            v.tensor_reduce(var[:], dd[:], AX.X, ALU.add)
            v.tensor_scalar_mul(var[:], var[:], 1.0 / (E - 1))
            # 1/sqrt(var) via exp(-0.5 ln var) + one Newton step
            lnv = gpool.tile([1, 1], F32)
            sc.activation(lnv[:], var[:], ACT.Ln)
            isd0 = gpool.tile([1, 1], F32)
            sc.activation(isd0[:], lnv[:], ACT.Exp, scale=-0.5)
            ii = gpool.tile([1, 1], F32)
            v.tensor_mul(ii[:], isd0[:], isd0[:])
            v.tensor_mul(ii[:], ii[:], var[:])
            v.tensor_scalar(ii[:], ii[:], -0.5, 1.5, ALU.mult, ALU.add)
            isd = gpool.tile([1, 1], F32)
            v.tensor_mul(isd[:], isd0[:], ii[:])
            scores = gpool.tile([1, E], F32)
            v.scalar_tensor_tensor(scores[:], d[:], isd[:], gl[:], ALU.mult, ALU.add)

            # rank each expert
            ranks = gpool.tile([1, E], F32)
            cmp = gpool.tile([1, E], F32)
            for e in range(E):
                v.tensor_scalar(
                    cmp[:], scores[:], scores[0:1, e:e + 1], None, ALU.is_gt
                )
                v.tensor_reduce(ranks[:, e:e + 1], cmp[:], AX.X, ALU.add)
            mask = gpool.tile([1, E], F32)
            v.tensor_scalar(mask[:], ranks[:], float(K), None, ALU.is_lt)

            # softmax over selected: gm = (gl+30)*mask - 30
            gm = gpool.tile([1, E], F32)
            v.scalar_tensor_tensor(gm[:], gl[:], 30.0, mask[:], ALU.add, ALU.mult)
            v.tensor_scalar_sub(gm[:], gm[:], 30.0)
            gmax = gpool.tile([1, 1], F32)
            v.tensor_reduce(gmax[:], gm[:], AX.X, ALU.max)
            ngmax = gpool.tile([1, 1], F32)
            v.tensor_scalar_mul(ngmax[:], gmax[:], -1.0)
            ex = gpool.tile([1, E], F32)
            sc.activation(ex[:], gm[:], ACT.Exp, bias=ngmax[:])
            ssum = gpool.tile([1, 1], F32)
            v.tensor_reduce(ssum[:], ex[:], AX.X, ALU.add)
            rs = gpool.tile([1, 1], F32)
            v.reciprocal(rs[:], ssum[:])

            # cat = [cof(6) | slotmask(18) | cof*slotmask(18)] on partition 0
            cat = gpool.tile([1, E + 2 * K * E], F32)
            cof = cat[:, 0:E]
            v.tensor_scalar(cof, ex[:], rs[:], None, ALU.mult)
            for s in range(K):
                sm = cat[:, E + s * E:E + (s + 1) * E]
                v.tensor_scalar(sm, ranks[:], float(s), None, ALU.is_equal)
                v.tensor_mul(cat[:, E + K * E + s * E:E + K * E + (s + 1) * E], sm, cof)

            # broadcast cat to all partitions via ones[1,C].T @ cat[1,42]
            ps_bc = psg_pool.tile([C, E + 2 * K * E], F32, tag="bc")
            nc.tensor.matmul(ps_bc[:], ones96[:], cat[:], start=True, stop=True)
            bc = cpool.tile([C, E + 2 * K * E], F32)
            v.tensor_copy(bc[:], ps_bc[:])

            def smask_ap(s, e):      # slot-mask broadcast column
                return bc[:, E + s * E + e:E + s * E + e + 1]

            def csmask_ap(s, e):     # cof * slot-mask broadcast column
                return bc[:, E + K * E + s * E + e:E + K * E + s * E + e + 1]

            # gather slot weights: w1s (f32), w2s (f32, cof-scaled), b1s
            w1s = cpool.tile([C, K * 9], F32)
            w2s = cpool.tile([C, K * 9], F32)
            b1s = cpool.tile([C, K], F32)
            for s in range(K):
                for e in range(E):
                    i0 = w1_sb[:, e * 9:(e + 1) * 9]
                    o0 = w1s[:, s * 9:(s + 1) * 9]
                    if e == 0:
                        v.tensor_scalar(o0, i0, smask_ap(s, e), None, ALU.mult)
                    else:
                        v.scalar_tensor_tensor(o0, i0, smask_ap(s, e), o0,
                                               ALU.mult, ALU.add)
                    i2 = w2_sb[:, e * 9:(e + 1) * 9]
                    o2 = w2s[:, s * 9:(s + 1) * 9]
                    if e == 0:
                        v.tensor_scalar(o2, i2, csmask_ap(s, e), None, ALU.mult)
                    else:
                        v.scalar_tensor_tensor(o2, i2, csmask_ap(s, e), o2,
                                               ALU.mult, ALU.add)
                    ib = b1_sb[:, e:e + 1]
                    ob = b1s[:, s:s + 1]
                    if e == 0:
                        v.tensor_scalar(ob, ib, smask_ap(s, e), None, ALU.mult)
                    else:
                        v.scalar_tensor_tensor(ob, ib, smask_ap(s, e), ob,
                                               ALU.mult, ALU.add)
            # b2tot = sum_e cof_e * b2_e
            tb = gpool.tile([C, E], F32)
            v.tensor_mul(tb[:], b2_sb[:], bc[:, 0:E])
            b2tot = cpool.tile([C, 1], F32)
            v.tensor_reduce(b2tot[:], tb[:], AX.X, ALU.add)

            # ---- fp8 weight split: w1s = w8 + dw8 (both exactly fp8) ------
            w8q = cpool.tile([C, K * 9], F8)
            sc.activation(w8q[:], w1s[:], ACT.Copy)
            w8f = cpool.tile([C, K * 9], F32)
            sc.activation(w8f[:], w8q[:], ACT.Copy)
            dw8q = cpool.tile([C, K * 9], F8)
            v.tensor_tensor(dw8q[:], w1s[:], w8f[:], ALU.subtract)
            dw8f = cpool.tile([C, K * 9], F32)
            sc.activation(dw8f[:], dw8q[:], ACT.Copy)

            # DoubleRow stationary matrices.
            # main (per slot, tap): [C, 2, C] fp8; plane0 = plane1 = diag(w8)
            # delta (per slot, tap-pair): plane0 = diag(dw8[t0]), plane1 =
            #   diag(dw8[t1]) (or zero for the unpaired 9th tap)
            PAIRS = [(0, 1), (2, 3), (4, 5), (6, 7), (8, None)]
            dr_main = []
            dr_delta = []
            for s in range(K):
                row_m = []
                row_d = []
                for t in range(9):
                    m = cpool.tile([C, 2 * C], F8, tag=f"drm{s}_{t}")
                    v.tensor_scalar(m[:], ee_sb[:], w8f[:, 9 * s + t:9 * s + t + 1],
                                    None, ALU.mult)
                    row_m.append(m)
                for pi, (t0, t1) in enumerate(PAIRS):
                    dm = cpool.tile([C, 2 * C], F8, tag=f"drd{s}_{pi}")
                    v.tensor_scalar(dm[:, 0:C], eye_sb[:],
                                    dw8f[:, 9 * s + t0:9 * s + t0 + 1],
                                    None, ALU.mult)
                    if t1 is None:
                        g.memset(dm[:, C:2 * C], 0.0)
                    else:
                        v.tensor_scalar(dm[:, C:2 * C], eye_sb[:],
                                        dw8f[:, 9 * s + t1:9 * s + t1 + 1],
                                        None, ALU.mult)
                    row_d.append(dm)
                dr_main.append(row_m)
                dr_delta.append(row_d)

            DELTAS = [dy * WP + dx for (dy, dx) in TAPS]

            # ---- pass B ---------------------------------------------------
            # Software-pipelined emission: conv2 ops of slot k are emitted
            # interleaved between conv1 psum-chunks of slot k+1 so the ACT
            # relu chain (which gates PE via psum buffers) is never stuck
            # behind a burst of conv2 muls, and DVE adds trail their ACT
            # producers.
            xi_flat = xi[:]
            achunks = []
            a0 = 0
            while a0 < HFLAT:
                achunks.append((a0, min(ACHUNK, HFLAT - a0)))
                a0 += ACHUNK

            from collections import deque
            pend = deque()
            pend_late = deque()

            def _late_ready():
                return pend_late and pend_late[0][0]()

            def drain_mix(n_now, n_late):
                for _ in range(n_now):
                    if pend:
                        pend.popleft()()
                for _ in range(n_late):
                    if _late_ready():
                        pend_late.popleft()[1]()

            def drain_all():
                while pend or pend_late:
                    if pend:
                        pend.popleft()()
                    if _late_ready():
                        pend_late.popleft()[1]()

            def emit_conv1_chunk(slot, h0, hf, c0, csz):
                ps = psc_pool.tile([C, ACHUNK], F32, tag="convps")
                for m0 in range(0, csz, MCHUNK):
                    msz = min(MCHUNK, csz - m0)
                    pix0 = h0 * WP + c0 + m0
                    out_ap = ps[:, m0:m0 + msz]
                    for t in range(9):
                        off = DBI + 2 * (pix0 + DELTAS[t])
                        rhs = xi_flat[:, off:off + 2 * msz].rearrange(
                            "c (n two) -> c two n", two=2)
                        nc.tensor.matmul(
                            out_ap,
                            dr_main[slot][t][:].rearrange("c (i j) -> c i j", i=2),
                            rhs,
                            start=(t == 0), stop=False,
                            perf_mode=mybir.MatmulPerfMode.DoubleRow,
                        )
                    for pi, (t0, t1) in enumerate(PAIRS):
                        d0 = DELTAS[t0]
                        dgap = 0 if t1 is None else (DELTAS[t1] - d0)
                        rhs = _raw_ap(xi_flat, DBI + 2 * (pix0 + d0),
                                      [[2 * dgap, 2], [2, msz]])
                        nc.tensor.matmul(
                            out_ap,
                            dr_delta[slot][pi][:].rearrange("c (i j) -> c i j", i=2),
                            rhs,
                            start=False, stop=(pi == len(PAIRS) - 1),
                            perf_mode=mybir.MatmulPerfMode.DoubleRow,
                        )
                sc.activation(hf[:, c0:c0 + csz], ps[:, 0:csz],
                              ACT.Relu, bias=b1s[:, slot:slot + 1])

            state = {}

            def queue_conv2(s, slot, ht):
                ctr = state["ctr"]
                st = state["st"]
                def dec(f):
                    def g():
                        f()
                        ctr[0] -= 1
                    return g
                for it, (dy, dx) in enumerate(TAPS):
                    in0 = ht[:, 1 + dy:1 + dy + TH, 1 + dx:1 + dx + W]
                    wap = w2s[:, slot * 9 + it:slot * 9 + it + 1]
                    lane = LANES[slot][it]
                    oacc, oaccp = state["oacc"], state["oaccp"]
                    if lane == 'd':
                        def f(in0=in0, wap=wap, oacc=oacc, st=st):
                            if st.pop("fd", None):
                                v.tensor_scalar(oacc[:], in0, wap, None, ALU.mult)
                            else:
                                p = t2_pool.tile([C, TH, W], BF, tag="tm")
                                v.tensor_scalar(p[:], in0, wap, None, ALU.mult)
                                v.tensor_add(oacc[:], oacc[:], p[:])
                        pend.append(dec(f))
                    elif lane == 'a':
                        box = {}
                        def fm(in0=in0, wap=wap, box=box):
                            p = t2_pool.tile([C, TH, W], BF, tag="ta")
                            sc.activation(p[:], in0, ACT.Copy, scale=wap)
                            box["p"] = p
                        def fa(box=box, oaccp=oaccp, st=st):
                            if st.pop("fp", None):
                                g.dma_start(oaccp[:], box["p"][:])
                            else:
                                g.dma_start(oaccp[:], box["p"][:],
                                            accum_op=ALU.add)
                        pend.append(fm)
                        pend_late.append((lambda box=box: "p" in box, dec(fa)))
                    elif lane == 'p':
                        def f(in0=in0, wap=wap, oaccp=oaccp, st=st):
                            if st.pop("fp", None):
                                g.tensor_scalar(oaccp[:], in0, wap, None, ALU.mult)
                            else:
                                p = t2_pool.tile([C, TH, W], BF, tag="tm")
                                g.tensor_scalar(p[:], in0, wap, None, ALU.mult)
                                g.tensor_add(oaccp[:], oaccp[:], p[:])
                        pend.append(dec(f))
                    else:  # 'm'
                        def f(in0=in0, wap=wap, oaccp=oaccp, st=st):
                            p = t2_pool.tile([C, TH, W], BF, tag="tm")
                            v.tensor_scalar(p[:], in0, wap, None, ALU.mult)
                            if st.pop("fp", None):
                                g.dma_start(oaccp[:], p[:])
                            else:
                                g.dma_start(oaccp[:], p[:], accum_op=ALU.add)
                        pend.append(dec(f))

            def queue_finish(s, h0):
                oacc, oaccp = state["oacc"], state["oaccp"]
                ctr = state["ctr"]
                def fmerge(oacc=oacc, oaccp=oaccp):
                    v.tensor_add(oacc[:], oacc[:], oaccp[:])
                pend_late.append((lambda ctr=ctr: ctr[0] == 0, fmerge))
                HH = TH // 2
                for hb in range(2):
                    def ffin(hb=hb, h0=h0, oacc=oacc):
                        of32 = of32_pool.tile([C, HH, W], F32, tag="of32")
                        sc.activation(of32[:], oacc[:, hb * HH:(hb + 1) * HH, :],
                                      ACT.Identity, bias=b2tot[:])
                        sy.dma_start(y[:, h0 + hb * HH:h0 + (hb + 1) * HH, :],
                                     of32[:])
                    pend_late.append((lambda ctr=ctr: ctr[0] == 0, ffin))

            for s in range(NS):
                h0 = s * TH
                state["oacc"] = oacc_pool.tile([C, TH, W], BF, tag="oacc_dve", name="oacc")
                state["oaccp"] = oacc_pool.tile([C, TH, W], BF, tag="oacc_pool", name="oaccp")
                state["st"] = {"fd": True, "fp": True}
                state["ctr"] = [27]
                for slot in range(K):
                    ht = h_pool.tile([C, TH + 2, WP], BF)
                    hf = ht[:].rearrange("c r w -> c (r w)")
                    for (c0, csz) in achunks:
                        emit_conv1_chunk(slot, h0, hf, c0, csz)
                        drain_mix(2, 1)
                    drain_mix(1, 1)
                    # zero h padding (cols, and top/bottom edge rows)
                    g.memset(ht[:, :, 0:1], 0.0)
                    g.memset(ht[:, :, WP - 1:WP], 0.0)
                    if s == 0:
                        g.memset(ht[:, 0:1, :], 0.0)
                    if s == NS - 1:
                        g.memset(ht[:, TH + 1:TH + 2, :], 0.0)
                    queue_conv2(s, slot, ht)
                queue_finish(s, h0)
                while len(pend) + len(pend_late) > 14:
                    drain_mix(1, 1)
            drain_all()

    if split:
        _split_multiwait(nc, maxw=1)
    return nc


_NC_CACHE = {}


def _get_nc():
    if "nc" not in _NC_CACHE:
        _NC_CACHE["nc"] = _build()
    return _NC_CACHE["nc"]


class _Runner:
    """Compile-once SPMD runner (mirrors bass2jax.run_bass_via_pjrt's
    multi-core path, but keeps the jitted executable for reuse/benching)."""

    def __init__(self, nc, n_cores):
        import jax
        from jax.experimental.shard_map import shard_map
        from jax.sharding import Mesh, PartitionSpec
        from concourse import bass2jax, mybir as _mybir

        bass2jax.install_neuronx_cc_hook()
        self.jax = jax
        partition_name = (
            nc.partition_id_tensor.name if nc.partition_id_tensor else None
        )
        in_names, out_names, out_avals, zero_outs = [], [], [], []
        for alloc in nc.m.functions[0].allocations:
            if not isinstance(alloc, _mybir.MemoryLocationSet):
                continue
            name = alloc.memorylocations[0].name
            if alloc.kind == "ExternalInput":
                if name == partition_name:
                    continue
                in_names.append(name)
            elif alloc.kind == "ExternalOutput":
                shape = tuple(alloc.tensor_shape)
                dtype = _mybir.dt.np(alloc.dtype)
                out_names.append(name)
                out_avals.append(jax.core.ShapedArray(shape, dtype))
                zero_outs.append(np.zeros(shape, dtype))
        self.in_names, self.out_names = in_names, out_names
        self.out_avals, self.zero_outs = out_avals, zero_outs
        n_params, n_outs = len(in_names), len(out_names)
        self.n_cores = n_cores
        donate = tuple(range(n_params, n_params + n_outs))

        all_in_names = in_names + out_names
        if partition_name is not None:
            all_in_names = all_in_names + [partition_name]

        def _body(*args):
            operands = list(args)
            if partition_name is not None:
                operands.append(bass2jax.partition_id_tensor())
            outs = bass2jax._bass_exec_p.bind(
                *operands,
                out_avals=tuple(out_avals),
                in_names=tuple(all_in_names),
                out_names=tuple(out_names),
                lowering_input_output_aliases=(),
                sim_require_finite=True,
                sim_require_nnan=True,
                nc=nc,
            )
            return tuple(outs)

        devices = jax.devices()[:n_cores]
        mesh = Mesh(np.asarray(devices), ("core",))
        self.sharded = jax.jit(
            shard_map(
                _body,
                mesh=mesh,
                in_specs=(PartitionSpec("core"),) * (n_params + n_outs),
                out_specs=(PartitionSpec("core"),) * n_outs,
                check_rep=False,
            ),
            donate_argnums=donate,
            keep_unused=True,
        )

    def concat_inputs(self, in_maps):
        return [
            np.concatenate([np.asarray(m[name]) for m in in_maps], axis=0)
            for name in self.in_names
        ]

    def concat_zeros(self):
        return [
            np.zeros((self.n_cores * z.shape[0], *z.shape[1:]), z.dtype)
            for z in self.zero_outs
        ]

    def run(self, in_maps):
        out_arrs = self.sharded(*self.concat_inputs(in_maps), *self.concat_zeros())
        return [
            {
                name: np.asarray(out_arrs[i]).reshape(
                    self.n_cores, *self.out_avals[i].shape
                )[c]
                for i, name in enumerate(self.out_names)
            }
            for c in range(self.n_cores)
        ]


def _get_runner():
    if "runner" not in _NC_CACHE:
        _NC_CACHE["runner"] = _Runner(_get_nc(), N_CORES)
    return _NC_CACHE["runner"]


_EYE = np.ascontiguousarray(np.eye(C, dtype=np.float32))
_EYEEYE = np.ascontiguousarray(np.concatenate([_EYE, _EYE], axis=1))


def make_in_maps(x, w_fc0, b_fc0, w_fc1, b_fc1, ew1, eb1, ew2, eb2):
    x = np.asarray(x, dtype=np.float32)
    f32 = lambda a: np.ascontiguousarray(np.asarray(a, dtype=np.float32))
    wfc = f32(np.concatenate([np.asarray(w_fc1).T, np.asarray(w_fc0).T], axis=1))
    bfc = f32(np.concatenate([np.asarray(b_fc1), np.asarray(b_fc0)])[None, :])
    w1p = f32(np.asarray(ew1).reshape(E, C, 9).transpose(1, 0, 2).reshape(C, E * 9))
    b1p = f32(np.asarray(eb1).T)
    w2p = f32(np.asarray(ew2).reshape(E, C, 9).transpose(1, 0, 2).reshape(C, E * 9))
    b2p = f32(np.asarray(eb2).T)

    in_maps = []
    for b in range(B):
        in_maps.append({
            "x": np.ascontiguousarray(x[b]),
            "wfc": wfc, "bfc": bfc,
            "w1": w1p, "b1": b1p, "w2": w2p, "b2": b2p,
            "eye": _EYE, "eyeeye": _EYEEYE,
        })
    return in_maps


def kernel(x, w_fc0, b_fc0, w_fc1, b_fc1, ew1, eb1, ew2, eb2):
    in_maps = make_in_maps(x, w_fc0, b_fc0, w_fc1, b_fc1, ew1, eb1, ew2, eb2)
    res = _get_runner().run(in_maps)
    out = np.stack([res[b]["y"] for b in range(B)], axis=0)
    return out.astype(np.float32)


if __name__ == "__main__":
    data = np.load("/tmp/ref_data.npz")
    inputs = {k: data[k] for k in
              ["x", "w_fc0", "b_fc0", "w_fc1", "b_fc1", "ew1", "eb1", "ew2", "eb2"]}
    out = kernel(**inputs)
    exp = data["out"]
    err = np.linalg.norm(out - exp) / np.linalg.norm(exp)
    print("Relative error:", err)
    print("max abs diff:", np.abs(out - exp).max())


# revision 18
# speedup vs baseline: 1.0838x; 1.0291x over previous
"""MoFE (mixture of depthwise-conv experts) Trainium2 kernel, v2.

Full inputs in, full outputs out; internally sharded data-parallel over the
batch dim across 8 NeuronCores (B=8, one sample per core).

Per-core program (Bass/Tile):
  pass A: stream x; per block: ACT casts to resident fp8 plane x8 (+row sums),
          Pool computes fp8 residual r8 = x - x8 (interleaved with x8), DVE
          max-reduces. Gate (pooled -> fc -> noisy top-k softmax) on device.
  conv1:  PE fp8 DoubleRow matmuls: 9 taps on (x8, r8) pairs + 5 tap-paired
          delta-weight correction matmuls -> f32 psum; ACT relu(+bias) -> h.
  conv2:  27 taps split across lanes: DVE mul+add pairs, ACT muls (+DVE adds),
          DVE muls + gpsimd accumulate-DMA adds, Pool mul+add pairs.
  merge:  DVE add of the two partial accumulators; ACT bias+f32; DMA store.
"""

import numpy as np

import concourse.bass as bass
import concourse.tile as tile
from concourse import mybir
from concourse.ap import AP as _AP

F32 = mybir.dt.float32
BF = mybir.dt.bfloat16
F8 = mybir.dt.float8e4
AX = mybir.AxisListType if hasattr(mybir, "AxisListType") else None
ALU = mybir.AluOpType
ACT = mybir.ActivationFunctionType

B, C, H, W = 8, 96, 192, 192
E = 6
N_CORES = 8
TH = 24                      # strip height (output rows per strip)
NS = H // TH                 # strips
TAPS = [(ky - 1, kx - 1) for ky in range(3) for kx in range(3)]
K = 3                        # top-k slots
WP = W + 2                   # padded width
NPAD = WP * WP
GUARD = 200                  # pixels of guard around the padded image
HFLAT = (TH + 2) * WP        # flat pixels per conv1 strip (incl halo rows)
ACHUNK = 1024                # psum chunk (2 banks) read at once by ACT
MCHUNK = 512                 # matmul psum sub-chunk (1 bank)

# conv2 lane schedule: per slot a list of 9 lane codes
#   'd' = DVE mul + DVE add, 'a' = ACT mul + DVE add,
#   'm' = DVE mul + gpsimd accumulate-DMA add, 'p' = Pool mul + Pool add
LANES = [
    ['m', 'm', 'a', 'd', 'd', 'a', 'd', 'a', 'd'],
    ['a', 'd', 'd', 'm', 'a', 'd', 'p', 'a', 'd'],
    ['a', 'd', 'd', 'm', 'a', 'd', 'm', 'a', 'd'],
]


# ---------------------------------------------------------------------------
# walrus workaround: split instructions carrying >maxw semaphore waits
# ---------------------------------------------------------------------------
def _split_multiwait(nc, maxw: int = 1) -> int:
    n_split = 0
    for f in nc.m.functions:
        for b in f.blocks:
            insts = b.instructions
            new_list = []
            changed = False
            for inst in insts:
                si = getattr(inst, "sync_info", None)
                waits = list(si.on_wait) if (si and si.on_wait) else []
                if len(waits) > maxw:
                    changed = True
                    chunks = [waits[j: j + maxw] for j in range(0, len(waits), maxw)]
                    for k, ch in enumerate(chunks[:-1]):
                        nop = mybir.InstNoOp(
                            name=f"{inst.name}_wsplit{k}",
                            sync_info=mybir.SyncInfo(on_wait=ch, on_update=[]),
                            bass_nofuse=True,
                            engine=inst.engine,
                        )
                        new_list.append(nop)
                        n_split += 1
                    si.on_wait = chunks[-1]
                new_list.append(inst)
            if changed:
                if isinstance(insts, list):
                    insts[:] = new_list
                else:
                    b.instructions = new_list
    return n_split


def _raw_ap(base, elem_off, dims):
    """Build a raw strided AP from a 2D tile AP `base` ([C, L] view):
    keeps the partition dim, replaces free dims with `dims` ([stride, count])
    and offsets by `elem_off` elements."""
    ap0 = [list(d) for d in base.ap]
    part = ap0[0]
    return _AP(base.tensor, base.offset + elem_off,
               [part] + [list(d) for d in dims])


# ---------------------------------------------------------------------------
# device program
# ---------------------------------------------------------------------------
def _build(split: bool = True):
    nc = bass.Bass()
    x = nc.declare_dram_parameter("x", [C, H, W], F32, isOutput=False)
    wfc = nc.declare_dram_parameter("wfc", [C, 2 * E], F32, isOutput=False)
    bfc = nc.declare_dram_parameter("bfc", [1, 2 * E], F32, isOutput=False)
    w1 = nc.declare_dram_parameter("w1", [C, E * 9], F32, isOutput=False)
    b1 = nc.declare_dram_parameter("b1", [C, E], F32, isOutput=False)
    w2 = nc.declare_dram_parameter("w2", [C, E * 9], F32, isOutput=False)
    b2 = nc.declare_dram_parameter("b2", [C, E], F32, isOutput=False)
    eye = nc.declare_dram_parameter("eye", [C, C], F32, isOutput=False)
    eyeeye = nc.declare_dram_parameter("eyeeye", [C, 2 * C], F32, isOutput=False)
    y = nc.declare_dram_parameter("y", [C, H, W], F32, isOutput=True)

    v = nc.vector
    g = nc.gpsimd
    sc = nc.scalar
    sy = nc.sync

    DBI = 2 * GUARD              # element offset of pixel 0 in xi (interleaved)
    XLEN = 2 * GUARD + 2 * NPAD + 2 * GUARD

    with tile.TileContext(nc) as tc:
        with (
            tc.tile_pool(name="const", bufs=1) as cpool,
            tc.tile_pool(name="gate", bufs=1) as gpool,
            tc.tile_pool(name="xa", bufs=3) as xa_pool,
            tc.tile_pool(name="hbuf", bufs=2) as h_pool,
            tc.tile_pool(name="tmp2", bufs=2) as t2_pool,
            tc.tile_pool(name="oacc", bufs=1) as oacc_pool,
            tc.tile_pool(name="of32", bufs=2) as of32_pool,
            tc.tile_pool(name="psg", bufs=1, space="PSUM") as psg_pool,
            tc.tile_pool(name="psc", bufs=3, space="PSUM") as psc_pool,
        ):
            # ---- constants ------------------------------------------------
            w1_sb = cpool.tile([C, E * 9], F32)
            sy.dma_start(w1_sb[:], w1[:])
            b1_sb = cpool.tile([C, E], F32)
            sy.dma_start(b1_sb[:], b1[:])
            w2_sb = cpool.tile([C, E * 9], F32)
            sy.dma_start(w2_sb[:], w2[:])
            b2_sb = cpool.tile([C, E], F32)
            sy.dma_start(b2_sb[:], b2[:])
            wfc_sb = cpool.tile([C, 2 * E], F32)
            sy.dma_start(wfc_sb[:], wfc[:])
            bfc_sb = cpool.tile([1, 2 * E], F32)
            sy.dma_start(bfc_sb[:], bfc[:])
            eye_sb = cpool.tile([C, C], F32)
            sy.dma_start(eye_sb[:], eye[:])
            ee_sb = cpool.tile([C, 2 * C], F32)
            sy.dma_start(ee_sb[:], eyeeye[:])
            ones96 = cpool.tile([1, C], F32)
            g.memset(ones96[:], 1.0)

            # resident interleaved fp8 image: even = x8, odd = r8 = x - x8
            xi = cpool.tile([C, XLEN], F8)
            # 4D interior view [c, row, col, plane] over the padded image
            xiI = xi[:, DBI:DBI + 2 * NPAD].rearrange(
                "c (r w two) -> c r w two", w=WP, two=2)
            # zero guards + padding ring (interleaved zeros are zeros)
            g.memset(xi[:, 0:DBI + 2 * WP], 0.0)
            g.memset(xi[:, DBI + 2 * (WP - 1) * WP:XLEN], 0.0)
            g.memset(xiI[:, 1:WP - 1, 0:1, :], 0.0)
            g.memset(xiI[:, 1:WP - 1, WP - 1:WP, :], 0.0)

            # ---- pass A: load f32; fp8 split; reduces ---------------------
            THA = 8
            NSA = H // THA
            maxbuf = gpool.tile([C, NSA], F32)
            sumbuf = gpool.tile([C, NSA], F32)
            for s in range(NSA):
                xa = xa_pool.tile([C, THA, W], F32)
                sy.dma_start(xa[:], x[:, s * THA:(s + 1) * THA, :])
                x8v = xiI[:, 1 + s * THA:1 + (s + 1) * THA, 1:W + 1, 0]
                r8v = xiI[:, 1 + s * THA:1 + (s + 1) * THA, 1:W + 1, 1]
                sc.activation(x8v, xa[:], ACT.Copy, accum_out=sumbuf[:, s:s + 1])
                # Pool recomputes its own fp8 cast so r8 does not wait on ACT
                t8 = xa_pool.tile([C, THA, W], F8, tag="t8")
                g.tensor_copy(t8[:], xa[:])
                g.tensor_tensor(r8v, xa[:], t8[:], ALU.subtract)
                v.tensor_reduce(maxbuf[:, s:s + 1], xa[:], AX.XY, ALU.max)
            maxv = gpool.tile([C, 1], F32)
            v.tensor_reduce(maxv[:], maxbuf[:], AX.X, ALU.max)
            sumv = gpool.tile([C, 1], F32)
            v.tensor_reduce(sumv[:], sumbuf[:], AX.X, ALU.add)
            pooled = gpool.tile([C, 1], F32)
            v.scalar_tensor_tensor(
                pooled[:], sumv[:], 1.0 / (H * W), maxv[:], ALU.mult, ALU.add
            )

            # ---- gate -----------------------------------------------------
            psg = psg_pool.tile([2 * E, 1], F32)
            nc.tensor.matmul(psg[:], wfc_sb[:], pooled[:], start=True, stop=True)
            g12 = gpool.tile([2 * E, 1], F32)
            v.tensor_copy(g12[:], psg[:])
            grow = gpool.tile([1, 2 * E], F32)
            sy.dma_start(grow[:], g12[:])          # partition -> free transpose
            gb = gpool.tile([1, 2 * E], F32)
            v.tensor_add(gb[:], grow[:], bfc_sb[:])
            g_pre = gb[:, 0:E]
            n_pre = gb[:, E:2 * E]

            # leaky relu(0.2)
            gl = gpool.tile([1, E], F32)
            t6 = gpool.tile([1, E], F32)
            v.tensor_scalar_mul(t6[:], g_pre, 0.2)
            v.tensor_max(gl[:], g_pre, t6[:])
            # softplus(x) = ln(1 + exp(x))
            e1 = gpool.tile([1, E], F32)
            sc.activation(e1[:], n_pre, ACT.Exp)
            noise = gpool.tile([1, E], F32)
            sc.activation(noise[:], e1[:], ACT.Ln, bias=1.0)
            # mean / unbiased std over experts
            mu = gpool.tile([1, 1], F32)
            v.tensor_reduce(mu[:], noise[:], AX.X, ALU.add)
            v.tensor_scalar_mul(mu[:], mu[:], 1.0 / E)
            d = gpool.tile([1, E], F32)
            v.tensor_scalar(d[:], noise[:], mu[:], None, ALU.subtract)
            dd = gpool.tile([1, E], F32)
            v.tensor_mul(dd[:], d[:], d[:])
            var = gpool.tile([1, 1], F32)
            v.tensor_reduce(var[:], dd[:], AX.X, ALU.add)
            v.tensor_scalar_mul(var[:], var[:], 1.0 / (E - 1))
            # 1/sqrt(var) via exp(-0.5 ln var) + one Newton step
            lnv = gpool.tile([1, 1], F32)
            sc.activation(lnv[:], var[:], ACT.Ln)
            isd0 = gpool.tile([1, 1], F32)
            sc.activation(isd0[:], lnv[:], ACT.Exp, scale=-0.5)
            ii = gpool.tile([1, 1], F32)
            v.tensor_mul(ii[:], isd0[:], isd0[:])
            v.tensor_mul(ii[:], ii[:], var[:])
            v.tensor_scalar(ii[:], ii[:], -0.5, 1.5, ALU.mult, ALU.add)
            isd = gpool.tile([1, 1], F32)
            v.tensor_mul(isd[:], isd0[:], ii[:])
            scores = gpool.tile([1, E], F32)
            v.scalar_tensor_tensor(scores[:], d[:], isd[:], gl[:], ALU.mult, ALU.add)

            # rank each expert
            ranks = gpool.tile([1, E], F32)
            cmp = gpool.tile([1, E], F32)
            for e in range(E):
                v.tensor_scalar(
                    cmp[:], scores[:], scores[0:1, e:e + 1], None, ALU.is_gt
                )
                v.tensor_reduce(ranks[:, e:e + 1], cmp[:], AX.X, ALU.add)
            mask = gpool.tile([1, E], F32)
            v.tensor_scalar(mask[:], ranks[:], float(K), None, ALU.is_lt)

            # softmax over selected: gm = (gl+30)*mask - 30
            gm = gpool.tile([1, E], F32)
            v.scalar_tensor_tensor(gm[:], gl[:], 30.0, mask[:], ALU.add, ALU.mult)
            v.tensor_scalar_sub(gm[:], gm[:], 30.0)
            gmax = gpool.tile([1, 1], F32)
            v.tensor_reduce(gmax[:], gm[:], AX.X, ALU.max)
            ngmax = gpool.tile([1, 1], F32)
            v.tensor_scalar_mul(ngmax[:], gmax[:], -1.0)
            ex = gpool.tile([1, E], F32)
            sc.activation(ex[:], gm[:], ACT.Exp, bias=ngmax[:])
            ssum = gpool.tile([1, 1], F32)
            v.tensor_reduce(ssum[:], ex[:], AX.X, ALU.add)
            rs = gpool.tile([1, 1], F32)
            v.reciprocal(rs[:], ssum[:])

            # cat = [cof(6) | slotmask(18) | cof*slotmask(18)] on partition 0
            cat = gpool.tile([1, E + 2 * K * E], F32)
            cof = cat[:, 0:E]
            v.tensor_scalar(cof, ex[:], rs[:], None, ALU.mult)
            for s in range(K):
                sm = cat[:, E + s * E:E + (s + 1) * E]
                v.tensor_scalar(sm, ranks[:], float(s), None, ALU.is_equal)
                v.tensor_mul(cat[:, E + K * E + s * E:E + K * E + (s + 1) * E], sm, cof)

            # broadcast cat to all partitions via ones[1,C].T @ cat[1,42]
            ps_bc = psg_pool.tile([C, E + 2 * K * E], F32, tag="bc")
            nc.tensor.matmul(ps_bc[:], ones96[:], cat[:], start=True, stop=True)
            bc = cpool.tile([C, E + 2 * K * E], F32)
            v.tensor_copy(bc[:], ps_bc[:])

            def smask_ap(s, e):      # slot-mask broadcast column
                return bc[:, E + s * E + e:E + s * E + e + 1]

            def csmask_ap(s, e):     # cof * slot-mask broadcast column
                return bc[:, E + K * E + s * E + e:E + K * E + s * E + e + 1]

            # gather slot weights: w1s (f32), w2s (f32, cof-scaled), b1s
            w1s = cpool.tile([C, K * 9], F32)
            w2s = cpool.tile([C, K * 9], F32)
            b1s = cpool.tile([C, K], F32)
            for s in range(K):
                for e in range(E):
                    i0 = w1_sb[:, e * 9:(e + 1) * 9]
                    o0 = w1s[:, s * 9:(s + 1) * 9]
                    if e == 0:
                        v.tensor_scalar(o0, i0, smask_ap(s, e), None, ALU.mult)
                    else:
                        v.scalar_tensor_tensor(o0, i0, smask_ap(s, e), o0,
                                               ALU.mult, ALU.add)
                    i2 = w2_sb[:, e * 9:(e + 1) * 9]
                    o2 = w2s[:, s * 9:(s + 1) * 9]
                    if e == 0:
                        v.tensor_scalar(o2, i2, csmask_ap(s, e), None, ALU.mult)
                    else:
                        v.scalar_tensor_tensor(o2, i2, csmask_ap(s, e), o2,
                                               ALU.mult, ALU.add)
                    ib = b1_sb[:, e:e + 1]
                    ob = b1s[:, s:s + 1]
                    if e == 0:
                        v.tensor_scalar(ob, ib, smask_ap(s, e), None, ALU.mult)
                    else:
                        v.scalar_tensor_tensor(ob, ib, smask_ap(s, e), ob,
                                               ALU.mult, ALU.add)
            # b2tot = sum_e cof_e * b2_e
            tb = gpool.tile([C, E], F32)
            v.tensor_mul(tb[:], b2_sb[:], bc[:, 0:E])
            b2tot = cpool.tile([C, 1], F32)
            v.tensor_reduce(b2tot[:], tb[:], AX.X, ALU.add)

            # ---- fp8 weight split: w1s = w8 + dw8 (both exactly fp8) ------
            w8q = cpool.tile([C, K * 9], F8)
            sc.activation(w8q[:], w1s[:], ACT.Copy)
            w8f = cpool.tile([C, K * 9], F32)
            sc.activation(w8f[:], w8q[:], ACT.Copy)
            dw8q = cpool.tile([C, K * 9], F8)
            v.tensor_tensor(dw8q[:], w1s[:], w8f[:], ALU.subtract)
            dw8f = cpool.tile([C, K * 9], F32)
            sc.activation(dw8f[:], dw8q[:], ACT.Copy)

            # DoubleRow stationary matrices.
            # main (per slot, tap): [C, 2, C] fp8; plane0 = plane1 = diag(w8)
            # delta (per slot, tap-pair): plane0 = diag(dw8[t0]), plane1 =
            #   diag(dw8[t1]) (or zero for the unpaired 9th tap)
            PAIRS = [(0, 1), (2, 3), (4, 5), (6, 7), (8, None)]
            dr_main = []
            dr_delta = []
            for s in range(K):
                row_m = []
                row_d = []
                for t in range(9):
                    m = cpool.tile([C, 2 * C], F8, tag=f"drm{s}_{t}")
                    v.tensor_scalar(m[:], ee_sb[:], w8f[:, 9 * s + t:9 * s + t + 1],
                                    None, ALU.mult)
                    row_m.append(m)
                for pi, (t0, t1) in enumerate(PAIRS):
                    dm = cpool.tile([C, 2 * C], F8, tag=f"drd{s}_{pi}")
                    v.tensor_scalar(dm[:, 0:C], eye_sb[:],
                                    dw8f[:, 9 * s + t0:9 * s + t0 + 1],
                                    None, ALU.mult)
                    if t1 is None:
                        g.memset(dm[:, C:2 * C], 0.0)
                    else:
                        v.tensor_scalar(dm[:, C:2 * C], eye_sb[:],
                                        dw8f[:, 9 * s + t1:9 * s + t1 + 1],
                                        None, ALU.mult)
                    row_d.append(dm)
                dr_main.append(row_m)
                dr_delta.append(row_d)

            DELTAS = [dy * WP + dx for (dy, dx) in TAPS]

            # ---- pass B ---------------------------------------------------
            # Software-pipelined emission: conv2 ops of slot k are emitted
            # interleaved between conv1 psum-chunks of slot k+1 so the ACT
            # relu chain (which gates PE via psum buffers) is never stuck
            # behind a burst of conv2 muls, and DVE adds trail their ACT
            # producers.
            xi_flat = xi[:]
            achunks = []
            a0 = 0
            while a0 < HFLAT:
                achunks.append((a0, min(ACHUNK, HFLAT - a0)))
                a0 += ACHUNK

            from collections import deque
            pend = deque()
            pend_late = deque()

            def _late_ready():
                return pend_late and pend_late[0][0]()

            def drain_mix(n_now, n_late):
                for _ in range(n_now):
                    if pend:
                        pend.popleft()()
                for _ in range(n_late):
                    if _late_ready():
                        pend_late.popleft()[1]()

            def drain_all():
                while pend or pend_late:
                    if pend:
                        pend.popleft()()
                    if _late_ready():
                        pend_late.popleft()[1]()

            def emit_conv1_chunk(slot, h0, hf, c0, csz):
                ps = psc_pool.tile([C, ACHUNK], F32, tag="convps")
                for m0 in range(0, csz, MCHUNK):
                    msz = min(MCHUNK, csz - m0)
                    pix0 = h0 * WP + c0 + m0
                    out_ap = ps[:, m0:m0 + msz]
                    for t in range(9):
                        off = DBI + 2 * (pix0 + DELTAS[t])
                        rhs = xi_flat[:, off:off + 2 * msz].rearrange(
                            "c (n two) -> c two n", two=2)
                        nc.tensor.matmul(
                            out_ap,
                            dr_main[slot][t][:].rearrange("c (i j) -> c i j", i=2),
                            rhs,
                            start=(t == 0), stop=False,
                            perf_mode=mybir.MatmulPerfMode.DoubleRow,
                        )
                    for pi, (t0, t1) in enumerate(PAIRS):
                        d0 = DELTAS[t0]
                        dgap = 0 if t1 is None else (DELTAS[t1] - d0)
                        rhs = _raw_ap(xi_flat, DBI + 2 * (pix0 + d0),
                                      [[2 * dgap, 2], [2, msz]])
                        nc.tensor.matmul(
                            out_ap,
                            dr_delta[slot][pi][:].rearrange("c (i j) -> c i j", i=2),
                            rhs,
                            start=False, stop=(pi == len(PAIRS) - 1),
                            perf_mode=mybir.MatmulPerfMode.DoubleRow,
                        )
                sc.activation(hf[:, c0:c0 + csz], ps[:, 0:csz],
                              ACT.Relu, bias=b1s[:, slot:slot + 1])

            state = {}

            def queue_conv2(s, slot, ht):
                ctr = state["ctr"]
                st = state["st"]
                def dec(f):
                    def g():
                        f()
                        ctr[0] -= 1
                    return g
                for it, (dy, dx) in enumerate(TAPS):
                    in0 = ht[:, 1 + dy:1 + dy + TH, 1 + dx:1 + dx + W]
                    wap = w2s[:, slot * 9 + it:slot * 9 + it + 1]
                    lane = LANES[slot][it]
                    oacc, oaccp = state["oacc"], state["oaccp"]
                    if lane == 'd':
                        def f(in0=in0, wap=wap, oacc=oacc, st=st):
                            if st.pop("fd", None):
                                v.tensor_scalar(oacc[:], in0, wap, None, ALU.mult)
                            else:
                                p = t2_pool.tile([C, TH, W], BF, tag="tm")
                                v.tensor_scalar(p[:], in0, wap, None, ALU.mult)
                                v.tensor_add(oacc[:], oacc[:], p[:])
                        pend.append(dec(f))
                    elif lane == 'a':
                        box = {}
                        def fm(in0=in0, wap=wap, box=box):
                            p = t2_pool.tile([C, TH, W], BF, tag="ta")
                            sc.activation(p[:], in0, ACT.Copy, scale=wap)
                            box["p"] = p
                        def fa(box=box, oaccp=oaccp, st=st):
                            if st.pop("fp", None):
                                g.dma_start(oaccp[:], box["p"][:])
                            else:
                                g.dma_start(oaccp[:], box["p"][:],
                                            accum_op=ALU.add)
                        pend.append(fm)
                        pend_late.append((lambda box=box: "p" in box, dec(fa)))
                    elif lane == 'p':
                        def f(in0=in0, wap=wap, oaccp=oaccp, st=st):
                            if st.pop("fp", None):
                                g.tensor_scalar(oaccp[:], in0, wap, None, ALU.mult)
                            else:
                                p = t2_pool.tile([C, TH, W], BF, tag="tm")
                                g.tensor_scalar(p[:], in0, wap, None, ALU.mult)
                                g.tensor_add(oaccp[:], oaccp[:], p[:])
                        pend.append(dec(f))
                    else:  # 'm'
                        def f(in0=in0, wap=wap, oaccp=oaccp, st=st):
                            p = t2_pool.tile([C, TH, W], BF, tag="tm")
                            v.tensor_scalar(p[:], in0, wap, None, ALU.mult)
                            if st.pop("fp", None):
                                g.dma_start(oaccp[:], p[:])
                            else:
                                g.dma_start(oaccp[:], p[:], accum_op=ALU.add)
                        pend.append(dec(f))

            def queue_finish(s, h0):
                oacc, oaccp = state["oacc"], state["oaccp"]
                ctr = state["ctr"]
                def fmerge(oacc=oacc, oaccp=oaccp):
                    v.tensor_add(oacc[:], oacc[:], oaccp[:])
                pend_late.append((lambda ctr=ctr: ctr[0] == 0, fmerge))
                HH = TH // 2
                for hb in range(2):
                    def ffin(hb=hb, h0=h0, oacc=oacc):
                        of32 = of32_pool.tile([C, HH, W], F32, tag="of32")
                        sc.activation(of32[:], oacc[:, hb * HH:(hb + 1) * HH, :],
                                      ACT.Identity, bias=b2tot[:])
                        sy.dma_start(y[:, h0 + hb * HH:h0 + (hb + 1) * HH, :],
                                     of32[:])
                    pend_late.append((lambda ctr=ctr: ctr[0] == 0, ffin))

            for s in range(NS):
                h0 = s * TH
                state["oacc"] = oacc_pool.tile([C, TH, W], BF, tag="oacc_dve", name="oacc")
                state["oaccp"] = oacc_pool.tile([C, TH, W], BF, tag="oacc_pool", name="oaccp")
                state["st"] = {"fd": True, "fp": True}
                state["ctr"] = [27]
                for slot in range(K):
                    ht = h_pool.tile([C, TH + 2, WP], BF)
                    hf = ht[:].rearrange("c r w -> c (r w)")
                    for (c0, csz) in achunks:
                        emit_conv1_chunk(slot, h0, hf, c0, csz)
                        drain_mix(2, 1)
                    drain_mix(1, 1)
                    # zero h padding (cols, and top/bottom edge rows)
                    g.memset(ht[:, :, 0:1], 0.0)
                    g.memset(ht[:, :, WP - 1:WP], 0.0)
                    if s == 0:
                        g.memset(ht[:, 0:1, :], 0.0)
                    if s == NS - 1:
                        g.memset(ht[:, TH + 1:TH + 2, :], 0.0)
                    queue_conv2(s, slot, ht)
                queue_finish(s, h0)
                while len(pend) + len(pend_late) > 8:
                    drain_mix(1, 1)
            drain_all()

    if split:
        _split_multiwait(nc, maxw=1)
    return nc


_NC_CACHE = {}


def _get_nc():
    if "nc" not in _NC_CACHE:
        _NC_CACHE["nc"] = _build()
    return _NC_CACHE["nc"]


class _Runner:
    """Compile-once SPMD runner (mirrors bass2jax.run_bass_via_pjrt's
    multi-core path, but keeps the jitted executable for reuse/benching)."""

    def __init__(self, nc, n_cores):
        import jax
        from jax.experimental.shard_map import shard_map
        from jax.sharding import Mesh, PartitionSpec
        from concourse import bass2jax, mybir as _mybir

        bass2jax.install_neuronx_cc_hook()
        self.jax = jax
        partition_name = (
            nc.partition_id_tensor.name if nc.partition_id_tensor else None
        )
        in_names, out_names, out_avals, zero_outs = [], [], [], []
        for alloc in nc.m.functions[0].allocations:
            if not isinstance(alloc, _mybir.MemoryLocationSet):
                continue
            name = alloc.memorylocations[0].name
            if alloc.kind == "ExternalInput":
                if name == partition_name:
                    continue
                in_names.append(name)
            elif alloc.kind == "ExternalOutput":
                shape = tuple(alloc.tensor_shape)
                dtype = _mybir.dt.np(alloc.dtype)
                out_names.append(name)
                out_avals.append(jax.core.ShapedArray(shape, dtype))
                zero_outs.append(np.zeros(shape, dtype))
        self.in_names, self.out_names = in_names, out_names
        self.out_avals, self.zero_outs = out_avals, zero_outs
        n_params, n_outs = len(in_names), len(out_names)
        self.n_cores = n_cores
        donate = tuple(range(n_params, n_params + n_outs))

        all_in_names = in_names + out_names
        if partition_name is not None:
            all_in_names = all_in_names + [partition_name]

        def _body(*args):
            operands = list(args)
            if partition_name is not None:
                operands.append(bass2jax.partition_id_tensor())
            outs = bass2jax._bass_exec_p.bind(
                *operands,
                out_avals=tuple(out_avals),
                in_names=tuple(all_in_names),
                out_names=tuple(out_names),
                lowering_input_output_aliases=(),
                sim_require_finite=True,
                sim_require_nnan=True,
                nc=nc,
            )
            return tuple(outs)

        devices = jax.devices()[:n_cores]
        mesh = Mesh(np.asarray(devices), ("core",))
        self.sharded = jax.jit(
            shard_map(
                _body,
                mesh=mesh,
                in_specs=(PartitionSpec("core"),) * (n_params + n_outs),
                out_specs=(PartitionSpec("core"),) * n_outs,
                check_rep=False,
            ),
            donate_argnums=donate,
            keep_unused=True,
        )

    def concat_inputs(self, in_maps):
        return [
            np.concatenate([np.asarray(m[name]) for m in in_maps], axis=0)
            for name in self.in_names
        ]

    def concat_zeros(self):
        return [
            np.zeros((self.n_cores * z.shape[0], *z.shape[1:]), z.dtype)
            for z in self.zero_outs
        ]

    def run(self, in_maps):
        out_arrs = self.sharded(*self.concat_inputs(in_maps), *self.concat_zeros())
        return [
            {
                name: np.asarray(out_arrs[i]).reshape(
                    self.n_cores, *self.out_avals[i].shape
                )[c]
                for i, name in enumerate(self.out_names)
            }
            for c in range(self.n_cores)
        ]


def _get_runner():
    if "runner" not in _NC_CACHE:
        _NC_CACHE["runner"] = _Runner(_get_nc(), N_CORES)
    return _NC_CACHE["runner"]


_EYE = np.ascontiguousarray(np.eye(C, dtype=np.float32))
_EYEEYE = np.ascontiguousarray(np.concatenate([_EYE, _EYE], axis=1))


def make_in_maps(x, w_fc0, b_fc0, w_fc1, b_fc1, ew1, eb1, ew2, eb2):
    x = np.asarray(x, dtype=np.float32)
    f32 = lambda a: np.ascontiguousarray(np.asarray(a, dtype=np.float32))
    wfc = f32(np.concatenate([np.asarray(w_fc1).T, np.asarray(w_fc0).T], axis=1))
    bfc = f32(np.concatenate([np.asarray(b_fc1), np.asarray(b_fc0)])[None, :])
    w1p = f32(np.asarray(ew1).reshape(E, C, 9).transpose(1, 0, 2).reshape(C, E * 9))
    b1p = f32(np.asarray(eb1).T)
    w2p = f32(np.asarray(ew2).reshape(E, C, 9).transpose(1, 0, 2).reshape(C, E * 9))
    b2p = f32(np.asarray(eb2).T)

    in_maps = []
    for b in range(B):
        in_maps.append({
            "x": np.ascontiguousarray(x[b]),
            "wfc": wfc, "bfc": bfc,
            "w1": w1p, "b1": b1p, "w2": w2p, "b2": b2p,
            "eye": _EYE, "eyeeye": _EYEEYE,
        })
    return in_maps


def kernel(x, w_fc0, b_fc0, w_fc1, b_fc1, ew1, eb1, ew2, eb2):
    in_maps = make_in_maps(x, w_fc0, b_fc0, w_fc1, b_fc1, ew1, eb1, ew2, eb2)
    res = _get_runner().run(in_maps)
    out = np.stack([res[b]["y"] for b in range(B)], axis=0)
    return out.astype(np.float32)


if __name__ == "__main__":
    data = np.load("/tmp/ref_data.npz")
    inputs = {k: data[k] for k in
              ["x", "w_fc0", "b_fc0", "w_fc1", "b_fc1", "ew1", "eb1", "ew2", "eb2"]}
    out = kernel(**inputs)
    exp = data["out"]
    err = np.linalg.norm(out - exp) / np.linalg.norm(exp)
    print("Relative error:", err)
    print("max abs diff:", np.abs(out - exp).max())


# revision 19
# speedup vs baseline: 1.1199x; 1.0333x over previous
"""MoFE (mixture of depthwise-conv experts) Trainium2 kernel, v2.

Full inputs in, full outputs out; internally sharded data-parallel over the
batch dim across 8 NeuronCores (B=8, one sample per core).

Per-core program (Bass/Tile):
  pass A: stream x; per block: ACT casts to resident fp8 plane x8 (+row sums),
          Pool computes fp8 residual r8 = x - x8 (interleaved with x8), DVE
          max-reduces. Gate (pooled -> fc -> noisy top-k softmax) on device.
  conv1:  PE fp8 DoubleRow matmuls: 9 taps on (x8, r8) pairs + 5 tap-paired
          delta-weight correction matmuls -> f32 psum; ACT relu(+bias) -> h.
  conv2:  27 taps split across lanes: DVE mul+add pairs, ACT muls (+DVE adds),
          DVE muls + gpsimd accumulate-DMA adds, Pool mul+add pairs.
  merge:  DVE add of the two partial accumulators; ACT bias+f32; DMA store.
"""

import numpy as np

import concourse.bass as bass
import concourse.tile as tile
from concourse import mybir
from concourse.ap import AP as _AP

F32 = mybir.dt.float32
BF = mybir.dt.bfloat16
F8 = mybir.dt.float8e4
AX = mybir.AxisListType if hasattr(mybir, "AxisListType") else None
ALU = mybir.AluOpType
ACT = mybir.ActivationFunctionType

B, C, H, W = 8, 96, 192, 192
E = 6
N_CORES = 8
TH = 24                      # strip height (output rows per strip)
NS = H // TH                 # strips
TAPS = [(ky - 1, kx - 1) for ky in range(3) for kx in range(3)]
K = 3                        # top-k slots
WP = W + 2                   # padded width
NPAD = WP * WP
GUARD = 200                  # pixels of guard around the padded image
HFLAT = (TH + 2) * WP        # flat pixels per conv1 strip (incl halo rows)
ACHUNK = 1024                # psum chunk (2 banks) read at once by ACT
MCHUNK = 512                 # matmul psum sub-chunk (1 bank)

# conv2 lane schedule: per slot a list of 9 lane codes
#   'd' = DVE mul + DVE add, 'a' = ACT mul + DVE add,
#   'm' = DVE mul + gpsimd accumulate-DMA add, 'p' = Pool mul + Pool add
LANES = [
    ['m', 'm', 'a', 'd', 'd', 'a', 'd', 'a', 'd'],
    ['a', 'd', 'd', 'm', 'a', 'd', 'p', 'a', 'd'],
    ['a', 'd', 'd', 'm', 'a', 'd', 'm', 'a', 'd'],
]


# ---------------------------------------------------------------------------
# walrus workaround: split instructions carrying >maxw semaphore waits
# ---------------------------------------------------------------------------
def _split_multiwait(nc, maxw: int = 1) -> int:
    n_split = 0
    for f in nc.m.functions:
        for b in f.blocks:
            insts = b.instructions
            new_list = []
            changed = False
            for inst in insts:
                si = getattr(inst, "sync_info", None)
                waits = list(si.on_wait) if (si and si.on_wait) else []
                if len(waits) > maxw:
                    changed = True
                    chunks = [waits[j: j + maxw] for j in range(0, len(waits), maxw)]
                    for k, ch in enumerate(chunks[:-1]):
                        nop = mybir.InstNoOp(
                            name=f"{inst.name}_wsplit{k}",
                            sync_info=mybir.SyncInfo(on_wait=ch, on_update=[]),
                            bass_nofuse=True,
                            engine=inst.engine,
                        )
                        new_list.append(nop)
                        n_split += 1
                    si.on_wait = chunks[-1]
                new_list.append(inst)
            if changed:
                if isinstance(insts, list):
                    insts[:] = new_list
                else:
                    b.instructions = new_list
    return n_split


def _raw_ap(base, elem_off, dims):
    """Build a raw strided AP from a 2D tile AP `base` ([C, L] view):
    keeps the partition dim, replaces free dims with `dims` ([stride, count])
    and offsets by `elem_off` elements."""
    ap0 = [list(d) for d in base.ap]
    part = ap0[0]
    return _AP(base.tensor, base.offset + elem_off,
               [part] + [list(d) for d in dims])


# ---------------------------------------------------------------------------
# device program
# ---------------------------------------------------------------------------
def _build(split: bool = True):
    nc = bass.Bass()
    x = nc.declare_dram_parameter("x", [C, H, W], F32, isOutput=False)
    wfc = nc.declare_dram_parameter("wfc", [C, 2 * E], F32, isOutput=False)
    bfc = nc.declare_dram_parameter("bfc", [1, 2 * E], F32, isOutput=False)
    w1 = nc.declare_dram_parameter("w1", [C, E * 9], F32, isOutput=False)
    b1 = nc.declare_dram_parameter("b1", [C, E], F32, isOutput=False)
    w2 = nc.declare_dram_parameter("w2", [C, E * 9], F32, isOutput=False)
    b2 = nc.declare_dram_parameter("b2", [C, E], F32, isOutput=False)
    eye = nc.declare_dram_parameter("eye", [C, C], F32, isOutput=False)
    eyeeye = nc.declare_dram_parameter("eyeeye", [C, 2 * C], F32, isOutput=False)
    y = nc.declare_dram_parameter("y", [C, H, W], F32, isOutput=True)

    v = nc.vector
    g = nc.gpsimd
    sc = nc.scalar
    sy = nc.sync

    DBI = 2 * GUARD              # element offset of pixel 0 in xi (interleaved)
    XLEN = 2 * GUARD + 2 * NPAD + 2 * GUARD

    with tile.TileContext(nc) as tc:
        with (
            tc.tile_pool(name="const", bufs=1) as cpool,
            tc.tile_pool(name="gate", bufs=1) as gpool,
            tc.tile_pool(name="hbuf", bufs=2) as h_pool,
            tc.tile_pool(name="tmp2", bufs=2) as t2_pool,
            tc.tile_pool(name="oacc", bufs=2) as oacc_pool,
            tc.tile_pool(name="of32", bufs=2) as of32_pool,
            tc.tile_pool(name="psg", bufs=1, space="PSUM") as psg_pool,
            tc.tile_pool(name="psc", bufs=3, space="PSUM") as psc_pool,
        ):
            # ---- constants ------------------------------------------------
            w1_sb = cpool.tile([C, E * 9], F32)
            sy.dma_start(w1_sb[:], w1[:])
            b1_sb = cpool.tile([C, E], F32)
            sy.dma_start(b1_sb[:], b1[:])
            w2_sb = cpool.tile([C, E * 9], F32)
            sy.dma_start(w2_sb[:], w2[:])
            b2_sb = cpool.tile([C, E], F32)
            sy.dma_start(b2_sb[:], b2[:])
            wfc_sb = cpool.tile([C, 2 * E], F32)
            sy.dma_start(wfc_sb[:], wfc[:])
            bfc_sb = cpool.tile([1, 2 * E], F32)
            sy.dma_start(bfc_sb[:], bfc[:])
            eye_sb = cpool.tile([C, C], F32)
            sy.dma_start(eye_sb[:], eye[:])
            ee_sb = cpool.tile([C, 2 * C], F32)
            sy.dma_start(ee_sb[:], eyeeye[:])
            ones96 = cpool.tile([1, C], F32)
            g.memset(ones96[:], 1.0)

            # resident interleaved fp8 image: even = x8, odd = r8 = x - x8
            xi = cpool.tile([C, XLEN], F8)
            # 4D interior view [c, row, col, plane] over the padded image
            xiI = xi[:, DBI:DBI + 2 * NPAD].rearrange(
                "c (r w two) -> c r w two", w=WP, two=2)
            # zero guards + padding ring (interleaved zeros are zeros)
            g.memset(xi[:, 0:DBI + 2 * WP], 0.0)
            g.memset(xi[:, DBI + 2 * (WP - 1) * WP:XLEN], 0.0)
            g.memset(xiI[:, 1:WP - 1, 0:1, :], 0.0)
            g.memset(xiI[:, 1:WP - 1, WP - 1:WP, :], 0.0)

            # ---- pass A: load f32; fp8 split; reduces ---------------------
            THA = 8
            NSA = H // THA
            maxbuf = gpool.tile([C, NSA], F32)
            sumbuf = gpool.tile([C, NSA], F32)
            # pass-A staging lives inside the pass-B accumulator/tmp slots
            # (bitcast views) so the SBUF is reused across phases.
            def _stage(s):
                pool, tag = [(oacc_pool, "oacc_dve"), (oacc_pool, "oacc_pool"),
                             (t2_pool, "ta"), (t2_pool, "tm")][s % 4]
                big = pool.tile([C, TH, W], BF, tag=tag, name=f"pa{s}")
                flat = big[:].rearrange("c r w -> c (r w)")
                xa = flat[:, 0:3072].bitcast(F32).rearrange(
                    "c (r w) -> c r w", w=W)
                t8 = flat[:, 3072:3840].bitcast(F8).rearrange(
                    "c (r w) -> c r w", w=W)
                return xa, t8
            for s in range(NSA):
                xa, t8 = _stage(s)
                sy.dma_start(xa, x[:, s * THA:(s + 1) * THA, :])
                x8v = xiI[:, 1 + s * THA:1 + (s + 1) * THA, 1:W + 1, 0]
                r8v = xiI[:, 1 + s * THA:1 + (s + 1) * THA, 1:W + 1, 1]
                sc.activation(x8v, xa, ACT.Copy, accum_out=sumbuf[:, s:s + 1])
                # Pool recomputes its own fp8 cast so r8 does not wait on ACT
                g.tensor_copy(t8, xa)
                g.tensor_tensor(r8v, xa, t8, ALU.subtract)
                v.tensor_reduce(maxbuf[:, s:s + 1], xa, AX.XY, ALU.max)
            maxv = gpool.tile([C, 1], F32)
            v.tensor_reduce(maxv[:], maxbuf[:], AX.X, ALU.max)
            sumv = gpool.tile([C, 1], F32)
            v.tensor_reduce(sumv[:], sumbuf[:], AX.X, ALU.add)
            pooled = gpool.tile([C, 1], F32)
            v.scalar_tensor_tensor(
                pooled[:], sumv[:], 1.0 / (H * W), maxv[:], ALU.mult, ALU.add
            )

            # ---- gate -----------------------------------------------------
            psg = psg_pool.tile([2 * E, 1], F32)
            nc.tensor.matmul(psg[:], wfc_sb[:], pooled[:], start=True, stop=True)
            g12 = gpool.tile([2 * E, 1], F32)
            v.tensor_copy(g12[:], psg[:])
            grow = gpool.tile([1, 2 * E], F32)
            sy.dma_start(grow[:], g12[:])          # partition -> free transpose
            gb = gpool.tile([1, 2 * E], F32)
            v.tensor_add(gb[:], grow[:], bfc_sb[:])
            g_pre = gb[:, 0:E]
            n_pre = gb[:, E:2 * E]

            # leaky relu(0.2)
            gl = gpool.tile([1, E], F32)
            t6 = gpool.tile([1, E], F32)
            v.tensor_scalar_mul(t6[:], g_pre, 0.2)
            v.tensor_max(gl[:], g_pre, t6[:])
            # softplus(x) = ln(1 + exp(x))
            e1 = gpool.tile([1, E], F32)
            sc.activation(e1[:], n_pre, ACT.Exp)
            noise = gpool.tile([1, E], F32)
            sc.activation(noise[:], e1[:], ACT.Ln, bias=1.0)
            # mean / unbiased std over experts
            mu = gpool.tile([1, 1], F32)
            v.tensor_reduce(mu[:], noise[:], AX.X, ALU.add)
            v.tensor_scalar_mul(mu[:], mu[:], 1.0 / E)
            d = gpool.tile([1, E], F32)
            v.tensor_scalar(d[:], noise[:], mu[:], None, ALU.subtract)
            dd = gpool.tile([1, E], F32)
            v.tensor_mul(dd[:], d[:], d[:])
            var = gpool.tile([1, 1], F32)
            v.tensor_reduce(var[:], dd[:], AX.X, ALU.add)
            v.tensor_scalar_mul(var[:], var[:], 1.0 / (E - 1))
            # 1/sqrt(var) via exp(-0.5 ln var) + one Newton step
            lnv = gpool.tile([1, 1], F32)
            sc.activation(lnv[:], var[:], ACT.Ln)
            isd0 = gpool.tile([1, 1], F32)
            sc.activation(isd0[:], lnv[:], ACT.Exp, scale=-0.5)
            ii = gpool.tile([1, 1], F32)
            v.tensor_mul(ii[:], isd0[:], isd0[:])
            v.tensor_mul(ii[:], ii[:], var[:])
            v.tensor_scalar(ii[:], ii[:], -0.5, 1.5, ALU.mult, ALU.add)
            isd = gpool.tile([1, 1], F32)
            v.tensor_mul(isd[:], isd0[:], ii[:])
            scores = gpool.tile([1, E], F32)
            v.scalar_tensor_tensor(scores[:], d[:], isd[:], gl[:], ALU.mult, ALU.add)

            # rank each expert
            ranks = gpool.tile([1, E], F32)
            cmp = gpool.tile([1, E], F32)
            for e in range(E):
                v.tensor_scalar(
                    cmp[:], scores[:], scores[0:1, e:e + 1], None, ALU.is_gt
                )
                v.tensor_reduce(ranks[:, e:e + 1], cmp[:], AX.X, ALU.add)
            mask = gpool.tile([1, E], F32)
            v.tensor_scalar(mask[:], ranks[:], float(K), None, ALU.is_lt)

            # softmax over selected: gm = (gl+30)*mask - 30
            gm = gpool.tile([1, E], F32)
            v.scalar_tensor_tensor(gm[:], gl[:], 30.0, mask[:], ALU.add, ALU.mult)
            v.tensor_scalar_sub(gm[:], gm[:], 30.0)
            gmax = gpool.tile([1, 1], F32)
            v.tensor_reduce(gmax[:], gm[:], AX.X, ALU.max)
            ngmax = gpool.tile([1, 1], F32)
            v.tensor_scalar_mul(ngmax[:], gmax[:], -1.0)
            ex = gpool.tile([1, E], F32)
            sc.activation(ex[:], gm[:], ACT.Exp, bias=ngmax[:])
            ssum = gpool.tile([1, 1], F32)
            v.tensor_reduce(ssum[:], ex[:], AX.X, ALU.add)
            rs = gpool.tile([1, 1], F32)
            v.reciprocal(rs[:], ssum[:])

            # cat = [cof(6) | slotmask(18) | cof*slotmask(18)] on partition 0
            cat = gpool.tile([1, E + 2 * K * E], F32)
            cof = cat[:, 0:E]
            v.tensor_scalar(cof, ex[:], rs[:], None, ALU.mult)
            for s in range(K):
                sm = cat[:, E + s * E:E + (s + 1) * E]
                v.tensor_scalar(sm, ranks[:], float(s), None, ALU.is_equal)
                v.tensor_mul(cat[:, E + K * E + s * E:E + K * E + (s + 1) * E], sm, cof)

            # broadcast cat to all partitions via ones[1,C].T @ cat[1,42]
            ps_bc = psg_pool.tile([C, E + 2 * K * E], F32, tag="bc")
            nc.tensor.matmul(ps_bc[:], ones96[:], cat[:], start=True, stop=True)
            bc = cpool.tile([C, E + 2 * K * E], F32)
            v.tensor_copy(bc[:], ps_bc[:])

            def smask_ap(s, e):      # slot-mask broadcast column
                return bc[:, E + s * E + e:E + s * E + e + 1]

            def csmask_ap(s, e):     # cof * slot-mask broadcast column
                return bc[:, E + K * E + s * E + e:E + K * E + s * E + e + 1]

            # gather slot weights: w1s (f32), w2s (f32, cof-scaled), b1s
            w1s = cpool.tile([C, K * 9], F32)
            w2s = cpool.tile([C, K * 9], F32)
            b1s = cpool.tile([C, K], F32)
            for s in range(K):
                for e in range(E):
                    i0 = w1_sb[:, e * 9:(e + 1) * 9]
                    o0 = w1s[:, s * 9:(s + 1) * 9]
                    if e == 0:
                        v.tensor_scalar(o0, i0, smask_ap(s, e), None, ALU.mult)
                    else:
                        v.scalar_tensor_tensor(o0, i0, smask_ap(s, e), o0,
                                               ALU.mult, ALU.add)
                    i2 = w2_sb[:, e * 9:(e + 1) * 9]
                    o2 = w2s[:, s * 9:(s + 1) * 9]
                    if e == 0:
                        v.tensor_scalar(o2, i2, csmask_ap(s, e), None, ALU.mult)
                    else:
                        v.scalar_tensor_tensor(o2, i2, csmask_ap(s, e), o2,
                                               ALU.mult, ALU.add)
                    ib = b1_sb[:, e:e + 1]
                    ob = b1s[:, s:s + 1]
                    if e == 0:
                        v.tensor_scalar(ob, ib, smask_ap(s, e), None, ALU.mult)
                    else:
                        v.scalar_tensor_tensor(ob, ib, smask_ap(s, e), ob,
                                               ALU.mult, ALU.add)
            # b2tot = sum_e cof_e * b2_e
            tb = gpool.tile([C, E], F32)
            v.tensor_mul(tb[:], b2_sb[:], bc[:, 0:E])
            b2tot = cpool.tile([C, 1], F32)
            v.tensor_reduce(b2tot[:], tb[:], AX.X, ALU.add)

            # ---- fp8 weight split: w1s = w8 + dw8 (both exactly fp8) ------
            w8q = cpool.tile([C, K * 9], F8)
            sc.activation(w8q[:], w1s[:], ACT.Copy)
            w8f = cpool.tile([C, K * 9], F32)
            sc.activation(w8f[:], w8q[:], ACT.Copy)
            dw8q = cpool.tile([C, K * 9], F8)
            v.tensor_tensor(dw8q[:], w1s[:], w8f[:], ALU.subtract)
            dw8f = cpool.tile([C, K * 9], F32)
            sc.activation(dw8f[:], dw8q[:], ACT.Copy)

            # DoubleRow stationary matrices.
            # main (per slot, tap): [C, 2, C] fp8; plane0 = plane1 = diag(w8)
            # delta (per slot, tap-pair): plane0 = diag(dw8[t0]), plane1 =
            #   diag(dw8[t1]) (or zero for the unpaired 9th tap)
            PAIRS = [(0, 1), (2, 3), (4, 5), (6, 7), (8, None)]
            dr_main = []
            dr_delta = []
            for s in range(K):
                row_m = []
                row_d = []
                for t in range(9):
                    m = cpool.tile([C, 2 * C], F8, tag=f"drm{s}_{t}")
                    v.tensor_scalar(m[:], ee_sb[:], w8f[:, 9 * s + t:9 * s + t + 1],
                                    None, ALU.mult)
                    row_m.append(m)
                for pi, (t0, t1) in enumerate(PAIRS):
                    dm = cpool.tile([C, 2 * C], F8, tag=f"drd{s}_{pi}")
                    v.tensor_scalar(dm[:, 0:C], eye_sb[:],
                                    dw8f[:, 9 * s + t0:9 * s + t0 + 1],
                                    None, ALU.mult)
                    if t1 is None:
                        g.memset(dm[:, C:2 * C], 0.0)
                    else:
                        v.tensor_scalar(dm[:, C:2 * C], eye_sb[:],
                                        dw8f[:, 9 * s + t1:9 * s + t1 + 1],
                                        None, ALU.mult)
                    row_d.append(dm)
                dr_main.append(row_m)
                dr_delta.append(row_d)

            DELTAS = [dy * WP + dx for (dy, dx) in TAPS]

            # ---- pass B ---------------------------------------------------
            # Software-pipelined emission: conv2 ops of slot k are emitted
            # interleaved between conv1 psum-chunks of slot k+1 so the ACT
            # relu chain (which gates PE via psum buffers) is never stuck
            # behind a burst of conv2 muls, and DVE adds trail their ACT
            # producers.
            xi_flat = xi[:]
            achunks = []
            a0 = 0
            while a0 < HFLAT:
                achunks.append((a0, min(ACHUNK, HFLAT - a0)))
                a0 += ACHUNK

            from collections import deque
            pend = deque()
            pend_late = deque()

            def _late_ready():
                return pend_late and pend_late[0][0]()

            def drain_mix(n_now, n_late):
                for _ in range(n_now):
                    if pend:
                        pend.popleft()()
                for _ in range(n_late):
                    if _late_ready():
                        pend_late.popleft()[1]()

            def drain_all():
                while pend or pend_late:
                    if pend:
                        pend.popleft()()
                    if _late_ready():
                        pend_late.popleft()[1]()

            def emit_conv1_chunk(slot, h0, hf, c0, csz):
                ps = psc_pool.tile([C, ACHUNK], F32, tag="convps")
                for m0 in range(0, csz, MCHUNK):
                    msz = min(MCHUNK, csz - m0)
                    pix0 = h0 * WP + c0 + m0
                    out_ap = ps[:, m0:m0 + msz]
                    for t in range(9):
                        off = DBI + 2 * (pix0 + DELTAS[t])
                        rhs = xi_flat[:, off:off + 2 * msz].rearrange(
                            "c (n two) -> c two n", two=2)
                        nc.tensor.matmul(
                            out_ap,
                            dr_main[slot][t][:].rearrange("c (i j) -> c i j", i=2),
                            rhs,
                            start=(t == 0), stop=False,
                            perf_mode=mybir.MatmulPerfMode.DoubleRow,
                        )
                    for pi, (t0, t1) in enumerate(PAIRS):
                        d0 = DELTAS[t0]
                        dgap = 0 if t1 is None else (DELTAS[t1] - d0)
                        rhs = _raw_ap(xi_flat, DBI + 2 * (pix0 + d0),
                                      [[2 * dgap, 2], [2, msz]])
                        nc.tensor.matmul(
                            out_ap,
                            dr_delta[slot][pi][:].rearrange("c (i j) -> c i j", i=2),
                            rhs,
                            start=False, stop=(pi == len(PAIRS) - 1),
                            perf_mode=mybir.MatmulPerfMode.DoubleRow,
                        )
                sc.activation(hf[:, c0:c0 + csz], ps[:, 0:csz],
                              ACT.Relu, bias=b1s[:, slot:slot + 1])

            state = {}

            def queue_conv2(s, slot, ht):
                ctr = state["ctr"]
                st = state["st"]
                def dec(f):
                    def g():
                        f()
                        ctr[0] -= 1
                    return g
                for it, (dy, dx) in enumerate(TAPS):
                    in0 = ht[:, 1 + dy:1 + dy + TH, 1 + dx:1 + dx + W]
                    wap = w2s[:, slot * 9 + it:slot * 9 + it + 1]
                    lane = LANES[slot][it]
                    oacc, oaccp = state["oacc"], state["oaccp"]
                    if lane == 'd':
                        def f(in0=in0, wap=wap, oacc=oacc, st=st):
                            if st.pop("fd", None):
                                v.tensor_scalar(oacc[:], in0, wap, None, ALU.mult)
                            else:
                                p = t2_pool.tile([C, TH, W], BF, tag="tm")
                                v.tensor_scalar(p[:], in0, wap, None, ALU.mult)
                                v.tensor_add(oacc[:], oacc[:], p[:])
                        pend.append(dec(f))
                    elif lane == 'a':
                        box = {}
                        def fm(in0=in0, wap=wap, box=box):
                            p = t2_pool.tile([C, TH, W], BF, tag="ta")
                            sc.activation(p[:], in0, ACT.Copy, scale=wap)
                            box["p"] = p
                        def fa(box=box, oaccp=oaccp, st=st):
                            if st.pop("fp", None):
                                g.dma_start(oaccp[:], box["p"][:])
                            else:
                                g.dma_start(oaccp[:], box["p"][:],
                                            accum_op=ALU.add)
                        pend.append(fm)
                        pend_late.append((lambda box=box: "p" in box, dec(fa)))
                    elif lane == 'p':
                        def f(in0=in0, wap=wap, oaccp=oaccp, st=st):
                            if st.pop("fp", None):
                                g.tensor_scalar(oaccp[:], in0, wap, None, ALU.mult)
                            else:
                                p = t2_pool.tile([C, TH, W], BF, tag="tm")
                                g.tensor_scalar(p[:], in0, wap, None, ALU.mult)
                                g.tensor_add(oaccp[:], oaccp[:], p[:])
                        pend.append(dec(f))
                    else:  # 'm'
                        def f(in0=in0, wap=wap, oaccp=oaccp, st=st):
                            p = t2_pool.tile([C, TH, W], BF, tag="tm")
                            v.tensor_scalar(p[:], in0, wap, None, ALU.mult)
                            if st.pop("fp", None):
                                g.dma_start(oaccp[:], p[:])
                            else:
                                g.dma_start(oaccp[:], p[:], accum_op=ALU.add)
                        pend.append(dec(f))

            def queue_finish(s, h0):
                oacc, oaccp = state["oacc"], state["oaccp"]
                ctr = state["ctr"]
                def fmerge(oacc=oacc, oaccp=oaccp):
                    v.tensor_add(oacc[:], oacc[:], oaccp[:])
                pend_late.append((lambda ctr=ctr: ctr[0] == 0, fmerge))
                HH = TH // 2
                for hb in range(2):
                    def ffin(hb=hb, h0=h0, oacc=oacc):
                        of32 = of32_pool.tile([C, HH, W], F32, tag="of32")
                        sc.activation(of32[:], oacc[:, hb * HH:(hb + 1) * HH, :],
                                      ACT.Identity, bias=b2tot[:])
                        sy.dma_start(y[:, h0 + hb * HH:h0 + (hb + 1) * HH, :],
                                     of32[:])
                    pend_late.append((lambda ctr=ctr: ctr[0] == 0, ffin))

            for s in range(NS):
                h0 = s * TH
                state["oacc"] = oacc_pool.tile([C, TH, W], BF, tag="oacc_dve", name="oacc")
                state["oaccp"] = oacc_pool.tile([C, TH, W], BF, tag="oacc_pool", name="oaccp")
                state["st"] = {"fd": True, "fp": True}
                state["ctr"] = [27]
                for slot in range(K):
                    ht = h_pool.tile([C, TH + 2, WP], BF)
                    hf = ht[:].rearrange("c r w -> c (r w)")
                    for (c0, csz) in achunks:
                        emit_conv1_chunk(slot, h0, hf, c0, csz)
                        drain_mix(2, 1)
                    drain_mix(1, 1)
                    # zero h padding (cols, and top/bottom edge rows)
                    g.memset(ht[:, :, 0:1], 0.0)
                    g.memset(ht[:, :, WP - 1:WP], 0.0)
                    if s == 0:
                        g.memset(ht[:, 0:1, :], 0.0)
                    if s == NS - 1:
                        g.memset(ht[:, TH + 1:TH + 2, :], 0.0)
                    queue_conv2(s, slot, ht)
                queue_finish(s, h0)
                while len(pend) + len(pend_late) > 8:
                    drain_mix(1, 1)
            drain_all()

    if split:
        _split_multiwait(nc, maxw=1)
    return nc


_NC_CACHE = {}


def _get_nc():
    if "nc" not in _NC_CACHE:
        _NC_CACHE["nc"] = _build()
    return _NC_CACHE["nc"]


class _Runner:
    """Compile-once SPMD runner (mirrors bass2jax.run_bass_via_pjrt's
    multi-core path, but keeps the jitted executable for reuse/benching)."""

    def __init__(self, nc, n_cores):
        import jax
        from jax.experimental.shard_map import shard_map
        from jax.sharding import Mesh, PartitionSpec
        from concourse import bass2jax, mybir as _mybir

        bass2jax.install_neuronx_cc_hook()
        self.jax = jax
        partition_name = (
            nc.partition_id_tensor.name if nc.partition_id_tensor else None
        )
        in_names, out_names, out_avals, zero_outs = [], [], [], []
        for alloc in nc.m.functions[0].allocations:
            if not isinstance(alloc, _mybir.MemoryLocationSet):
                continue
            name = alloc.memorylocations[0].name
            if alloc.kind == "ExternalInput":
                if name == partition_name:
                    continue
                in_names.append(name)
            elif alloc.kind == "ExternalOutput":
                shape = tuple(alloc.tensor_shape)
                dtype = _mybir.dt.np(alloc.dtype)
                out_names.append(name)
                out_avals.append(jax.core.ShapedArray(shape, dtype))
                zero_outs.append(np.zeros(shape, dtype))
        self.in_names, self.out_names = in_names, out_names
        self.out_avals, self.zero_outs = out_avals, zero_outs
        n_params, n_outs = len(in_names), len(out_names)
        self.n_cores = n_cores
        donate = tuple(range(n_params, n_params + n_outs))

        all_in_names = in_names + out_names
        if partition_name is not None:
            all_in_names = all_in_names + [partition_name]

        def _body(*args):
            operands = list(args)
            if partition_name is not None:
                operands.append(bass2jax.partition_id_tensor())
            outs = bass2jax._bass_exec_p.bind(
                *operands,
                out_avals=tuple(out_avals),
                in_names=tuple(all_in_names),
                out_names=tuple(out_names),
                lowering_input_output_aliases=(),
                sim_require_finite=True,
                sim_require_nnan=True,
                nc=nc,
            )
            return tuple(outs)

        devices = jax.devices()[:n_cores]
        mesh = Mesh(np.asarray(devices), ("core",))
        self.sharded = jax.jit(
            shard_map(
                _body,
                mesh=mesh,
                in_specs=(PartitionSpec("core"),) * (n_params + n_outs),
                out_specs=(PartitionSpec("core"),) * n_outs,
                check_rep=False,
            ),
            donate_argnums=donate,
            keep_unused=True,
        )

    def concat_inputs(self, in_maps):
        return [
            np.concatenate([np.asarray(m[name]) for m in in_maps], axis=0)
            for name in self.in_names
        ]

    def concat_zeros(self):
        return [
            np.zeros((self.n_cores * z.shape[0], *z.shape[1:]), z.dtype)
            for z in self.zero_outs
        ]

    def run(self, in_maps):
        out_arrs = self.sharded(*self.concat_inputs(in_maps), *self.concat_zeros())
        return [
            {
                name: np.asarray(out_arrs[i]).reshape(
                    self.n_cores, *self.out_avals[i].shape
                )[c]
                for i, name in enumerate(self.out_names)
            }
            for c in range(self.n_cores)
        ]


def _get_runner():
    if "runner" not in _NC_CACHE:
        _NC_CACHE["runner"] = _Runner(_get_nc(), N_CORES)
    return _NC_CACHE["runner"]


_EYE = np.ascontiguousarray(np.eye(C, dtype=np.float32))
_EYEEYE = np.ascontiguousarray(np.concatenate([_EYE, _EYE], axis=1))


def make_in_maps(x, w_fc0, b_fc0, w_fc1, b_fc1, ew1, eb1, ew2, eb2):
    x = np.asarray(x, dtype=np.float32)
    f32 = lambda a: np.ascontiguousarray(np.asarray(a, dtype=np.float32))
    wfc = f32(np.concatenate([np.asarray(w_fc1).T, np.asarray(w_fc0).T], axis=1))
    bfc = f32(np.concatenate([np.asarray(b_fc1), np.asarray(b_fc0)])[None, :])
    w1p = f32(np.asarray(ew1).reshape(E, C, 9).transpose(1, 0, 2).reshape(C, E * 9))
    b1p = f32(np.asarray(eb1).T)
    w2p = f32(np.asarray(ew2).reshape(E, C, 9).transpose(1, 0, 2).reshape(C, E * 9))
    b2p = f32(np.asarray(eb2).T)

    in_maps = []
    for b in range(B):
        in_maps.append({
            "x": np.ascontiguousarray(x[b]),
            "wfc": wfc, "bfc": bfc,
            "w1": w1p, "b1": b1p, "w2": w2p, "b2": b2p,
            "eye": _EYE, "eyeeye": _EYEEYE,
        })
    return in_maps


def kernel(x, w_fc0, b_fc0, w_fc1, b_fc1, ew1, eb1, ew2, eb2):
    in_maps = make_in_maps(x, w_fc0, b_fc0, w_fc1, b_fc1, ew1, eb1, ew2, eb2)
    res = _get_runner().run(in_maps)
    out = np.stack([res[b]["y"] for b in range(B)], axis=0)
    return out.astype(np.float32)


if __name__ == "__main__":
    data = np.load("/tmp/ref_data.npz")
    inputs = {k: data[k] for k in
              ["x", "w_fc0", "b_fc0", "w_fc1", "b_fc1", "ew1", "eb1", "ew2", "eb2"]}
    out = kernel(**inputs)
    exp = data["out"]
    err = np.linalg.norm(out - exp) / np.linalg.norm(exp)
    print("Relative error:", err)
    print("max abs diff:", np.abs(out - exp).max())
